# revision 12
# baseline (speedup 1.0000x reference)
"""LinearRNN final-state kernel for 8 Trainium2 NeuronCores.

Reference computation:
    u_t = Wxh @ x_t + bxh            (input projection)
    h_t = u_t + Whh @ h_{t-1}        (recurrence over T=1024 steps)
    return h_T                        -> [B=32, H=512]

The recurrence is linear:  h_T = sum_t u_t @ A^(T-1-t),  A = Whh^T
(row-vector convention).  This kernel evaluates it as:

  * radix-4 fused projection: 4 consecutive timesteps fold directly in the
    input GEMM via the matrices  W, WA, WA^2, WA^3  (W = Wxh^T), absorbing
    the first two tree levels;
  * a per-block binary tree: 32-timestep blocks collapse 8 -> 4 -> 2 -> 1
    with A^4, A^8, A^16;
  * an interleaved Horner chain over the 32 blocks with stationary A^32:
    h <- h @ A^32 + V_k.  Chain rounds hide behind the next blocks' tree
    work, so only A^2..A^32 (5 matrix squarings) are ever materialized
    instead of the 9 a full scan tree needs.
  * the bias rides the chain as a closed-form 5th column:
    c32 = b (I + A + ... + A^31), added once per round.

Everything on the PE runs in bf16 (1 cycle/row at any width; empirically
rel-err ~5e-3 vs the 2e-2 budget).  x is converted f32->bf16 on the Act
engine and transposed by the DMA xbar engine (14ns/tile), so the PE does
no transpose work for x at all.

Sharding: data-parallel over batch (B=32 -> 4 rows/core on 8 cores);
weights and the squaring chain are replicated.
"""

import numpy as np

B, T, IN, H = 32, 1024, 256, 512
NCORES = 8
BC = B // NCORES          # 4 batch rows per core
COLS = BC * T             # 4096 sequence columns per core
HC = H // 128             # 4 hidden-dim chunks of 128
ICH = IN // 128           # 2 input-dim chunks
L = 32                    # timesteps per Horner block
NBLK = T // L             # 32 blocks, each spanning all BC rows
NG = COLS // 512          # 8 x-groups of 512 columns

_cache: dict = {}


def _build():
    import concourse.bass as bass
    import concourse.mybir as mybir
    from concourse import bacc
    from concourse.tile import TileContext
    from concourse.masks import make_identity

    f32 = mybir.dt.float32
    bf16 = mybir.dt.bfloat16
    ACT_COPY = mybir.ActivationFunctionType.Copy

    nc = bacc.Bacc(None)
    x_d = nc.declare_dram_parameter("x", [COLS, IN], f32, isOutput=False)
    wxh_d = nc.declare_dram_parameter("Wxh", [H, IN], f32, isOutput=False)
    bxh_d = nc.declare_dram_parameter("bxh", [H], f32, isOutput=False)
    whh_d = nc.declare_dram_parameter("Whh", [H, H], f32, isOutput=False)
    # Output stays in on-chip layout [128, HC*BC]; host unscrambles.
    out_d = nc.declare_dram_parameter("h_out", [128, HC * BC], f32, isOutput=True)

    with TileContext(nc) as tc:
        with (
            tc.tile_pool(name="const", bufs=1) as cpool,
            tc.tile_pool(name="xload", bufs=4) as xpool,
            tc.tile_pool(name="mats", bufs=2) as spool,
            tc.tile_pool(name="blk", bufs=3) as bpool,
            tc.tile_pool(name="mm", bufs=2, space="PSUM") as mmpool,
            tc.tile_pool(name="tr", bufs=2, space="PSUM") as trpool,
            tc.tile_pool(name="bp", bufs=3, space="PSUM") as bppool,
        ):
            ident_b = cpool.tile([128, 128], bf16, tag="identb")
            make_identity(nc, ident_b[:])

            # PE warm-up: keeps the PE busy through the initial weight-DMA
            # wait and completes the clock ramp before real work arrives.
            warm = mmpool.tile([128, 512], f32, tag="mm")
            for _ in range(24):
                nc.tensor.matmul(
                    warm[:, 0:128], ident_b[:], ident_b[:], start=True, stop=True
                )

            # ---- loads -------------------------------------------------
            w_f32 = cpool.tile([128, HC, H], f32, tag="wf32")
            nc.scalar.dma_start(w_f32[:], whh_d.rearrange("(c p) f -> p c f", p=128))
            wxh_f32 = cpool.tile([128, HC, IN], f32, tag="wxf32")
            nc.scalar.dma_start(
                wxh_f32[:], wxh_d.rearrange("(c p) f -> p c f", p=128)
            )
            b_f32 = cpool.tile([128, HC], f32, tag="bf32")
            nc.scalar.dma_start(b_f32[:], bxh_d.rearrange("(c p) -> p c", p=128))

            # x groups; order matters: the first 16 blocks need groups
            # 0,2,4,6 (first half of every batch row).
            load_order = [0, 2, 4, 6, 1, 3, 5, 7]
            xg_f32 = {}
            for g in load_order:
                xg = xpool.tile([128, 4, IN], f32, tag="xg")
                nc.sync.dma_start(
                    xg[:],
                    x_d[g * 512:(g + 1) * 512, :].rearrange(
                        "(j p) i -> p j i", p=128
                    ),
                )
                xg_f32[g] = xg

            # ---- bf16 conversions of weights --------------------------
            w_bf = cpool.tile([128, HC, H], bf16, tag="wbf")  # Whh = A^T natural
            for c in range(HC):
                nc.vector.tensor_copy(w_bf[:, c, :], w_f32[:, c, :])
            wxh_bf = cpool.tile([128, HC, IN], bf16, tag="wxbf")
            for c in range(0, HC, 2):
                nc.scalar.activation(
                    wxh_bf[:, c:c + 2, :], wxh_f32[:, c:c + 2, :], ACT_COPY
                )
            b_bf = cpool.tile([128, HC, 1], bf16, tag="bbf")
            nc.scalar.activation(b_bf[:, :, 0], b_f32[:], ACT_COPY)

            def transpose_quad(dst_ap, srcs, engine="dve"):
                """Transpose up to four [128,128] bf16 blocks into one PSUM
                tile, then move them out with a single wide copy (bf16
                transposes run at 1 cycle/row)."""
                tp = trpool.tile([128, 4, 128], bf16, tag="tp")
                for i, s in enumerate(srcs):
                    nc.tensor.transpose(tp[:, i, :], s, ident_b[:])
                if engine == "act":
                    nc.scalar.activation(dst_ap, tp[:, :len(srcs), :], ACT_COPY)
                else:
                    nc.vector.tensor_copy(dst_ap, tp[:, :len(srcs), :])

            # S1 = A natural: S1[p, kc, f] = A[kc*128+p, f] = Whh[f, kc*128+p]
            S1 = cpool.tile([128, HC, H], bf16, tag="S1")
            for kc in range(HC):
                transpose_quad(
                    S1[:, kc, :],
                    [w_bf[:, rc, kc * 128:(kc + 1) * 128] for rc in range(HC)],
                )
            # Wq = Wxh^T as lhsT: Wq[p, ic, h] = Wxh[h, ic*128+p]
            Wq = cpool.tile([128, ICH, H], bf16, tag="Wq")
            for ic in range(ICH):
                transpose_quad(
                    Wq[:, ic, :],
                    [wxh_bf[:, rc, ic * 128:(ic + 1) * 128] for rc in range(HC)],
                )

            def square(U, S, name, copy_eng="dve"):
                """A^{2m} = mm(lhsT=U=A^m^T, rhs=S=A^m), natural layout."""
                Snew = spool.tile([128, HC, H], bf16, tag=name, bufs=1)
                for mcc in range(HC):
                    ps = mmpool.tile([128, H], f32, tag="mm")
                    for kc in range(HC):
                        nc.tensor.matmul(
                            ps[:],
                            U[:, kc, mcc * 128:(mcc + 1) * 128],
                            S[:, kc, :],
                            start=(kc == 0),
                            stop=(kc == HC - 1),
                        )
                    if copy_eng == "act":
                        nc.scalar.activation(Snew[:, mcc, :], ps[:], ACT_COPY)
                    else:
                        nc.vector.tensor_copy(Snew[:, mcc, :], ps[:])
                return Snew

            def transpose_mat(S, name):
                """U[p, kc, f] = S[f, kc*128+p] (natural -> lhsT-for-squaring)."""
                U = spool.tile([128, HC, H], bf16, tag=name, bufs=2)
                for kc in range(HC):
                    transpose_quad(
                        U[:, kc, :],
                        [S[:, fc, kc * 128:(kc + 1) * 128] for fc in range(HC)],
                    )
                return U

            def c_round(c_prev, S_m, name):
                """c_{2m} = c_m + c_m @ A^m  (column form, A^m = S_m)."""
                ps = bppool.tile([128, HC, 1], f32, tag="cps", bufs=1)
                for mcc in range(HC):
                    for kc in range(HC):
                        nc.tensor.matmul(
                            ps[:, mcc, :],
                            S_m[:, kc, mcc * 128:(mcc + 1) * 128],
                            c_prev[:, kc, :],
                            start=(kc == 0),
                            stop=(kc == HC - 1),
                        )
                c_new = cpool.tile([128, HC, 1], bf16, tag=name)
                nc.vector.tensor_add(c_new[:], ps[:], c_prev[:])
                return c_new

            def g_mats(lhsT_tile, lhs_chunks, lhs_slice, rhs, out_ch, name,
                       width, copy_eng="act"):
                """out[:, oc, :] = sum_kc mm(lhsT chunk, rhs chunk)."""
                G = cpool.tile([128, out_ch, width], bf16, tag=name)
                for oc in range(out_ch):
                    ps = mmpool.tile([128, H], f32, tag="mm")
                    for kc in range(lhs_chunks):
                        nc.tensor.matmul(
                            ps[:, 0:width],
                            lhs_slice(lhsT_tile, kc, oc),
                            rhs(kc),
                            start=(kc == 0),
                            stop=(kc == lhs_chunks - 1),
                        )
                    if copy_eng == "act":
                        nc.scalar.activation(G[:, oc, :], ps[:, 0:width], ACT_COPY)
                    else:
                        nc.vector.tensor_copy(G[:, oc, :], ps[:, 0:width])
                return G

            # ---- x path: convert to bf16 (Act), transpose via DMA xbar -
            # Emitted FIRST on the Act ring so the conversions (which pend
            # on the x DMAs) sit ahead of the squaring-chain copies.
            # xT_all[q, g, gb, p] = x_bf[512 g + (gb//2)*128 + p,
            #                            (gb%2)*128 + q]
            xT_all = cpool.tile([128, NG, 8, 128], bf16, tag="xT")
            xg_bf = {}

            def emit_conv(g):
                xb = xpool.tile([128, 4, IN], bf16, tag="xgbf")
                nc.scalar.activation(xb[:], xg_f32[g][:], ACT_COPY)
                xg_bf[g] = xb

            def emit_xpose(g):
                nc.sync.dma_start_transpose(xT_all[:, g, :, :], xg_bf[g][:])

            for g in (0, 2, 4, 6):
                emit_conv(g)
            for g in (0, 2, 4, 6):
                emit_xpose(g)

            # ---- squaring chain + G matrices + c vector ----------------
            # Per level: transposes of the previous power, then filler
            # matmuls (G's / c rounds) while the copies land, then the
            # squaring matmuls.
            # A^2 (U1 = Whh natural = A^T in lhsT layout)
            S2 = square(w_bf, S1, "S2")
            c2 = c_round(b_bf, S1, "c2")
            # G1 = W A  (lhsT = Wxh natural chunks of wxh_bf)
            G1 = g_mats(
                wxh_bf, HC,
                lambda t, kc, oc: t[:, kc, oc * 128:(oc + 1) * 128],
                lambda kc: S1[:, kc, :],
                ICH, "G1", H,
            )
            U2 = transpose_mat(S2, "U")
            c4 = c_round(c2, S2, "c4")
            # G2 = W A^2
            G2 = g_mats(
                wxh_bf, HC,
                lambda t, kc, oc: t[:, kc, oc * 128:(oc + 1) * 128],
                lambda kc: S2[:, kc, :],
                ICH, "G2", H,
            )
            # G1T[p, hc, i] = G1[i, hc*128+p]  (for G3)
            G1T = g_mats(
                S1, HC,
                lambda t, kc, oc: t[:, kc, oc * 128:(oc + 1) * 128],
                lambda kc: wxh_bf[:, kc, :],
                HC, "G1T", IN,
            )
            S4 = square(U2, S2, "S4")
            U4 = transpose_mat(S4, "U")
            c8 = c_round(c4, S4, "c8")
            # G3 = G1 A^2 = W A^3
            G3 = g_mats(
                G1T, HC,
                lambda t, kc, oc: t[:, kc, oc * 128:(oc + 1) * 128],
                lambda kc: S2[:, kc, :],
                ICH, "G3", H,
            )
            S8 = square(U4, S4, "S8")
            U8 = transpose_mat(S8, "U")
            c16 = c_round(c8, S8, "c16")
            S16 = square(U8, S8, "S16")
            U16 = transpose_mat(S16, "U")
            c32 = c_round(c16, S16, "c32")
            S32 = square(U16, S16, "S32", copy_eng="act")

            # ---- main loop: per-block trees + interleaved Horner -------
            # Block k: timesteps [32k, 32k+32), all 4 batch rows.
            # half = k//16 selects the x-group parity; within the group the
            # block sits at sub-block sb = (k%16)//4, partitions
            # p0 = 32*(k%4) .. p0+32.
            PROJ_MATS = [Wq, G1, G2, G3]  # applied to t ≡ 3,2,1,0 (mod 4)

            def rhs_proj(k, m, ic):
                half, sb, p0 = k // 16, (k % 16) // 4, 32 * (k % 4)
                gb = 2 * sb + ic
                return xT_all[:, half::2, gb, p0 + 3 - m:p0 + 32:4]

            v0s, v1s, v2s, hs = {}, {}, {}, {}

            blkps = {}

            def emit_proj(k):
                # one PSUM bank per in-flight block; stages carve slices
                ps = bppool.tile([128, HC, 64], f32, tag="blkps")
                blkps[k] = ps
                pp = ps[:, :, 0:32]
                for mcc in range(HC):
                    first = True
                    for m in range(4):
                        for ic in range(ICH):
                            nc.tensor.matmul(
                                pp[:, mcc, :],
                                PROJ_MATS[m][:, ic, mcc * 128:(mcc + 1) * 128],
                                rhs_proj(k, m, ic),
                                start=first,
                                stop=(m == 3 and ic == ICH - 1),
                            )
                            first = False
                v0 = bpool.tile([128, HC, 32], bf16, tag="v0")
                nc.scalar.activation(v0[:], pp[:], ACT_COPY)
                v0s[k] = v0

            def emit_l2(k):
                v0r = v0s[k][:].rearrange("p c (b j) -> p c b j", b=BC)
                p2 = blkps[k][:, :, 32:48]
                for mcc in range(HC):
                    for kc in range(HC):
                        nc.tensor.matmul(
                            p2[:, mcc, :],
                            S4[:, kc, mcc * 128:(mcc + 1) * 128],
                            v0r[:, kc, :, 0::2],
                            start=(kc == 0),
                            stop=(kc == HC - 1),
                        )
                v1 = bpool.tile([128, HC, 16], bf16, tag="v1")
                nc.vector.tensor_add(
                    v1[:].rearrange("p c (b j) -> p c b j", b=BC),
                    p2[:].rearrange("p c (b j) -> p c b j", b=BC),
                    v0r[:, :, :, 1::2],
                )
                v1s[k] = v1
                del v0s[k]

            def emit_l3(k):
                v1r = v1s[k][:].rearrange("p c (b j) -> p c b j", b=BC)
                p3 = blkps[k][:, :, 48:56]
                for mcc in range(HC):
                    for kc in range(HC):
                        nc.tensor.matmul(
                            p3[:, mcc, :],
                            S8[:, kc, mcc * 128:(mcc + 1) * 128],
                            v1r[:, kc, :, 0::2],
                            start=(kc == 0),
                            stop=(kc == HC - 1),
                        )
                v2 = bpool.tile([128, HC, 8], bf16, tag="v2")
                nc.vector.tensor_add(
                    v2[:].rearrange("p c (b j) -> p c b j", b=BC),
                    p3[:].rearrange("p c (b j) -> p c b j", b=BC),
                    v1r[:, :, :, 1::2],
                )
                v2s[k] = v2
                del v1s[k]

            def emit_l4_horner(k):
                # One PSUM group per mcc (open/close sequentially): the l4
                # tree part accumulates cols 0:4, then the Horner part
                # (h_{k-1} @ A^32) accumulates cols 0:5 (bias column rides
                # as col 4).
                v2r = v2s[k][:].rearrange("p c (b j) -> p c b j", b=BC)
                p4 = blkps[k][:, :, 56:64]
                hp = hs[k - 1] if k > 0 else None
                for mcc in range(HC):
                    for kc in range(HC):
                        nc.tensor.matmul(
                            p4[:, mcc, 0:4],
                            S16[:, kc, mcc * 128:(mcc + 1) * 128],
                            v2r[:, kc, :, 0],
                            start=(kc == 0),
                            stop=(k == 0 and kc == HC - 1),
                        )
                    if k > 0:
                        for kc in range(HC):
                            nc.tensor.matmul(
                                p4[:, mcc, 0:5],
                                S32[:, kc, mcc * 128:(mcc + 1) * 128],
                                hp[:, kc, :],
                                start=False,
                                stop=(kc == HC - 1),
                            )
                h = bpool.tile([128, HC, 5], bf16, tag="h")
                nc.vector.tensor_add(
                    h[:, :, 0:4], p4[:, :, 0:4], v2r[:, :, :, 1]
                )
                if k > 0:
                    nc.vector.tensor_add(h[:, :, 4:5], p4[:, :, 4:5], c32[:])
                else:
                    nc.gpsimd.tensor_copy(h[:, :, 4:5], c32[:])
                hs[k] = h
                if k > 0:
                    del hs[k - 1]
                del v2s[k]
                del blkps[k]

            stages = [emit_proj, emit_l2, emit_l3, emit_l4_horner]
            for step in range(NBLK + 3):
                if step == 10:
                    for g in (1, 3, 5, 7):
                        emit_conv(g)
                    for g in (1, 3, 5, 7):
                        emit_xpose(g)
                for si, fn in enumerate(stages):
                    k = step - si
                    if 0 <= k < NBLK:
                        fn(k)

            # ---- finalize: add the bias column into each batch column --
            hT = hs[NBLK - 1]
            fout = cpool.tile([128, HC, BC], f32, tag="fout")
            for b in range(BC):
                nc.vector.tensor_add(
                    fout[:, :, b:b + 1], hT[:, :, b:b + 1], hT[:, :, 4:5]
                )
            nc.sync.dma_start(
                out_d.rearrange("p (c b) -> p c b", b=BC), fout[:]
            )

    nc.compile()
    return nc


def _get_nc():
    if "nc" not in _cache:
        _cache["nc"] = _build()
    return _cache["nc"]


def _in_maps(inputs):
    x = np.ascontiguousarray(np.asarray(inputs["x"], dtype=np.float32))
    wxh = np.ascontiguousarray(np.asarray(inputs["Wxh"], dtype=np.float32))
    bxh = np.ascontiguousarray(np.asarray(inputs["bxh"], dtype=np.float32))
    whh = np.ascontiguousarray(np.asarray(inputs["Whh"], dtype=np.float32))
    return [
        dict(
            x=np.ascontiguousarray(
                x[c * BC:(c + 1) * BC].reshape(COLS, IN)
            ),
            Wxh=wxh,
            bxh=bxh,
            Whh=whh,
        )
        for c in range(NCORES)
    ]


def kernel(**inputs) -> np.ndarray:
    from concourse.bass_utils import run_bass_kernel_spmd

    res = run_bass_kernel_spmd(
        _get_nc(), _in_maps(inputs), list(range(NCORES))
    ).results
    return _assemble(res)


def _assemble(results) -> np.ndarray:
    outs = []
    for c in range(NCORES):
        o = np.asarray(results[c]["h_out"])      # [128, HC*BC] on-chip layout
        o = o.reshape(128, HC, BC).transpose(2, 1, 0).reshape(BC, H)
        outs.append(o)
    return np.concatenate(outs, axis=0).astype(np.float32)


# revision 16
# speedup vs baseline: 1.2106x; 1.2106x over previous
"""LinearRNN final-state kernel for 8 Trainium2 NeuronCores.

Reference computation:
    u_t = Wxh @ x_t + bxh            (input projection)
    h_t = u_t + Whh @ h_{t-1}        (recurrence over T=1024 steps)
    return h_T                        -> [B=32, H=512]

The recurrence is linear:  h_T = sum_t u_t @ A^(T-1-t),  A = Whh^T
(row-vector convention).  Structure (driven by the ~58ns/instruction PE
floor: matmuls below ~139 moving columns are instruction-bound):

  * radix-4 fused projection: 4 consecutive timesteps fold directly into
    the input GEMM via W, WA, WA^2, WA^3 (W = Wxh^T), absorbing the first
    two tree levels; the bias enters here as b(I+A+A^2+A^3).
  * wide per-half tree levels with A^4..A^64 collapse each half of the
    sequence to 4 blocks of 128 timesteps per batch row (matmuls stay
    >=64 wide);
  * a Horner chain over the 8 blocks with stationary A^128:
    h <- h @ A^128 + V_k.  Rounds 1-3 (first-half blocks) hide behind the
    second half's projection work.
  * only A^2..A^128 are materialized (7 squaring products); their
    lhsT-layout transposes ride the DMA xbar engine (14ns per 16x128
    tile) except the first two, which sit on the setup critical path and
    use PE quad-transposes.

Everything on the PE runs in bf16 (1 cycle/row at any width; empirically
rel-err ~6e-3 vs the 2e-2 budget).  x is converted f32->bf16 on Act and
transposed by the DMA xbar, so the PE does no transpose work for x.

Sharding: data-parallel over batch (B=32 -> 4 rows/core on 8 cores);
weights and the squaring chain are replicated.
"""

import numpy as np

B, T, IN, H = 32, 1024, 256, 512
NCORES = 8
BC = B // NCORES          # 4 batch rows per core
COLS = BC * T             # 4096 sequence columns per core
HC = H // 128             # 4 hidden-dim chunks of 128
ICH = IN // 128           # 2 input-dim chunks
NG = COLS // 512          # 8 x-groups (each = half of one batch row)

_cache: dict = {}


def _build():
    import concourse.bass as bass
    import concourse.mybir as mybir
    from concourse import bacc
    from concourse.tile import TileContext
    from concourse.masks import make_identity

    f32 = mybir.dt.float32
    bf16 = mybir.dt.bfloat16
    ACT_COPY = mybir.ActivationFunctionType.Copy
    ACT_IDENT = mybir.ActivationFunctionType.Identity

    nc = bacc.Bacc(None)
    x_d = nc.declare_dram_parameter("x", [COLS, IN], f32, isOutput=False)
    wxh_d = nc.declare_dram_parameter("Wxh", [H, IN], f32, isOutput=False)
    bxh_d = nc.declare_dram_parameter("bxh", [H], f32, isOutput=False)
    whh_d = nc.declare_dram_parameter("Whh", [H, H], f32, isOutput=False)
    # Output stays in on-chip layout [128, HC*BC]; host unscrambles.
    out_d = nc.declare_dram_parameter("h_out", [128, HC * BC], f32, isOutput=True)

    with TileContext(nc) as tc:
        with (
            tc.tile_pool(name="const", bufs=1) as cpool,
            tc.tile_pool(name="xload", bufs=4) as xpool,
            tc.tile_pool(name="vbuf", bufs=1) as vpool,
            tc.tile_pool(name="mm", bufs=2, space="PSUM") as mmpool,
            tc.tile_pool(name="tr", bufs=1, space="PSUM") as trpool,
            tc.tile_pool(name="pj", bufs=2, space="PSUM") as pjpool,
            tc.tile_pool(name="tl", bufs=2, space="PSUM") as tlpool,
            tc.tile_pool(name="sm", bufs=1, space="PSUM") as smpool,
        ):
            ident_b = cpool.tile([128, 128], bf16, tag="identb")
            make_identity(nc, ident_b[:])

            # PE warm-up: keeps the PE busy through the initial weight-DMA
            # wait and completes the clock ramp before real work arrives.
            warm = mmpool.tile([128, H], f32, tag="mm")
            for _ in range(24):
                nc.tensor.matmul(
                    warm[:, 0:128], ident_b[:], ident_b[:], start=True, stop=True
                )

            # ---- loads -------------------------------------------------
            w_f32 = cpool.tile([128, HC, H], f32, tag="wf32")
            nc.scalar.dma_start(w_f32[:], whh_d.rearrange("(c p) f -> p c f", p=128))
            wxh_f32 = cpool.tile([128, HC, IN], f32, tag="wxf32")
            nc.scalar.dma_start(
                wxh_f32[:], wxh_d.rearrange("(c p) f -> p c f", p=128)
            )
            b_f32 = cpool.tile([128, HC], f32, tag="bf32")
            nc.scalar.dma_start(b_f32[:], bxh_d.rearrange("(c p) -> p c", p=128))

            # x groups; group g = half (g%2) of batch row g//2.  The first
            # half of every row is needed first.
            load_order = [0, 2, 4, 6, 1, 3, 5, 7]
            xg_f32 = {}
            for g in load_order:
                xg = xpool.tile([128, 4, IN], f32, tag="xg")
                nc.sync.dma_start(
                    xg[:],
                    x_d[g * 512:(g + 1) * 512, :].rearrange(
                        "(j p) i -> p j i", p=128
                    ),
                )
                xg_f32[g] = xg

            # ---- bf16 conversions of weights --------------------------
            w_bf = cpool.tile([128, HC, H], bf16, tag="wbf")  # Whh = A^T natural
            for c in range(HC):
                nc.vector.tensor_copy(w_bf[:, c, :], w_f32[:, c, :])
            wxh_bf = cpool.tile([128, HC, IN], bf16, tag="wxbf")
            nc.scalar.activation(wxh_bf[:], wxh_f32[:], ACT_COPY)
            b_bf = cpool.tile([128, HC, 1], bf16, tag="bbf")
            nc.scalar.activation(b_bf[:, :, 0], b_f32[:], ACT_COPY)

            # ---- x path: convert to bf16 (Act), transpose via DMA xbar -
            # xT_all[q, g, gb, p] = x_bf[512 g + (gb//2)*128 + p,
            #                            (gb%2)*128 + q]
            xT_all = cpool.tile([128, NG, 8, 128], bf16, tag="xT")
            xg_bf = {}

            def emit_conv(g):
                xb = xpool.tile([128, 4, IN], bf16, tag="xgbf")
                nc.scalar.activation(xb[:], xg_f32[g][:], ACT_COPY)
                xg_bf[g] = xb

            def emit_xpose(g):
                nc.sync.dma_start_transpose(xT_all[:, g, :, :], xg_bf[g][:])

            for g in (0, 2, 4, 6):
                emit_conv(g)
            for g in (0, 2, 4, 6):
                emit_xpose(g)

            def transpose_quad(dst_ap, srcs):
                """PE transpose of up to four [128,128] bf16 blocks through
                one PSUM tile + a single wide DVE copy."""
                tp = trpool.tile([128, 4, 128], bf16, tag="tp")
                for i, s in enumerate(srcs):
                    nc.tensor.transpose(tp[:, i, :], s, ident_b[:])
                nc.vector.tensor_copy(dst_ap, tp[:, :len(srcs), :])

            # S1 = A natural: S1[p, kc, f] = A[kc*128+p, f] = Whh[f, kc*128+p]
            S1 = cpool.tile([128, HC, H], bf16, tag="S1")
            for kc in range(HC):
                transpose_quad(
                    S1[:, kc, :],
                    [w_bf[:, rc, kc * 128:(kc + 1) * 128] for rc in range(HC)],
                )
            # Wq = Wxh^T as lhsT: Wq[p, ic, h] = Wxh[h, ic*128+p]
            Wq = cpool.tile([128, ICH, H], bf16, tag="Wq")
            for ic in range(ICH):
                transpose_quad(
                    Wq[:, ic, :],
                    [wxh_bf[:, rc, ic * 128:(ic + 1) * 128] for rc in range(HC)],
                )

            # lhsT accessors: natural [128, HC, H] tiles slice
            # [:, kc, mcc-block]; DMA-transposed tiles are [128, 16, 128]
            # and slice [:, 4*mcc + kc, :].
            def nat(tile):
                return lambda kc, mcc: tile[:, kc, mcc * 128:(mcc + 1) * 128]

            def xbar(tile):
                return lambda kc, mcc: tile[:, 4 * mcc + kc, :]

            def square(U_sl, S, name, copy_eng="dve"):
                """A^{2m}: out[:, mcc, :] = sum_kc U_sl(kc,mcc)^T @ S[:,kc,:]."""
                Snew = cpool.tile([128, HC, H], bf16, tag=name)
                for mcc in range(HC):
                    ps = mmpool.tile([128, H], f32, tag="mm")
                    for kc in range(HC):
                        nc.tensor.matmul(
                            ps[:],
                            U_sl(kc, mcc),
                            S[:, kc, :],
                            start=(kc == 0),
                            stop=(kc == HC - 1),
                        )
                    if copy_eng == "act":
                        nc.scalar.activation(Snew[:, mcc, :], ps[:], ACT_COPY)
                    else:
                        nc.vector.tensor_copy(Snew[:, mcc, :], ps[:])
                return Snew

            def pe_transpose_mat(S, name):
                U = cpool.tile([128, HC, H], bf16, tag=name)
                for kc in range(HC):
                    transpose_quad(
                        U[:, kc, :],
                        [S[:, fc, kc * 128:(kc + 1) * 128] for fc in range(HC)],
                    )
                return nat(U)

            def dma_transpose_mat(S, name):
                U = cpool.tile([128, 16, 128], bf16, tag=name)
                nc.scalar.dma_start_transpose(U[:], S[:])
                return xbar(U)

            def c_round(c_prev, S_m, name):
                """c_{2m} = c_m + c_m @ A^m  (column form)."""
                psf = smpool.tile([128, HC, 4], f32, tag="sm", name="csm")
                ps = psf[:, :, 0:1]
                for mcc in range(HC):
                    for kc in range(HC):
                        nc.tensor.matmul(
                            ps[:, mcc, :],
                            S_m[:, kc, mcc * 128:(mcc + 1) * 128],
                            c_prev[:, kc, :],
                            start=(kc == 0),
                            stop=(kc == HC - 1),
                        )
                c_new = cpool.tile([128, HC, 1], bf16, tag=name)
                nc.vector.tensor_add(c_new[:], ps[:], c_prev[:])
                return c_new

            def g_mats(lhs_sl, rhs, name):
                """G[:, oc, :] = sum_kc lhs_sl(kc,oc)^T @ rhs(kc), 512 wide."""
                G = cpool.tile([128, ICH, H], bf16, tag=name)
                for oc in range(ICH):
                    ps = mmpool.tile([128, H], f32, tag="mm")
                    for kc in range(HC):
                        nc.tensor.matmul(
                            ps[:],
                            lhs_sl(kc, oc),
                            rhs(kc),
                            start=(kc == 0),
                            stop=(kc == HC - 1),
                        )
                    nc.scalar.activation(G[:, oc, :], ps[:], ACT_COPY)
                return G

            # ---- squaring chain / G matrices / bias vector -------------
            S2 = square(nat(w_bf), S1, "S2")        # U1 = Whh natural
            c2 = c_round(b_bf, S1, "c2")
            # G1 = W A  (lhsT chunks = Wxh natural rows of wxh_bf)
            G1 = g_mats(
                lambda kc, oc: wxh_bf[:, kc, oc * 128:(oc + 1) * 128],
                lambda kc: S1[:, kc, :], "G1",
            )
            U2 = pe_transpose_mat(S2, "U2")
            c4 = c_round(c2, S2, "c4")              # = b(I+A+A^2+A^3)
            G2 = g_mats(
                lambda kc, oc: wxh_bf[:, kc, oc * 128:(oc + 1) * 128],
                lambda kc: S2[:, kc, :], "G2",
            )
            S4 = square(U2, S2, "S4")
            # G1T[p, ·, ·] = G1 transposed via DMA xbar (for G3)
            G1T = cpool.tile([128, 8, 128], bf16, tag="G1T")
            nc.scalar.dma_start_transpose(G1T[:], G1[:])
            U4 = pe_transpose_mat(S4, "U4")
            # G3 = G1 A^2 = W A^3
            G3 = g_mats(
                lambda kc, oc: G1T[:, 4 * oc + kc, :],
                lambda kc: S2[:, kc, :], "G3",
            )
            S8 = square(U4, S4, "S8")
            U8 = dma_transpose_mat(S8, "U8")
            S16 = square(U8, S8, "S16")
            U16 = dma_transpose_mat(S16, "U16")
            S32 = square(U16, S16, "S32")
            U32 = dma_transpose_mat(S32, "U32")
            S64 = square(U32, S32, "S64")
            U64 = dma_transpose_mat(S64, "U64")
            S128 = square(U64, S64, "S128", copy_eng="act")

            PROJ_MATS = [Wq, G1, G2, G3]  # applied to t ≡ 3,2,1,0 (mod 4)

            # ---- main pipeline -----------------------------------------
            # Per x-group (512 timesteps of one batch row): radix-4 fused
            # projection -> 128 quad-columns.  Per half (4 groups, one per
            # row): tree levels A^4..A^64 collapse 512 -> 16 columns
            # (4 blocks of 128 timesteps x 4 rows, b-major).  A Horner
            # chain joins the 8 blocks with A^128.
            vhalf = {}
            v5 = {}

            def emit_proj(g):
                half, row = g % 2, g // 2
                if half not in vhalf:
                    vhalf[half] = vpool.tile(
                        [128, HC, 512], bf16, tag=f"v0h{half}",
                        name=f"v0h{half}",
                    )
                pp = pjpool.tile([128, HC, 128], f32, tag="pj")
                for mcc in range(HC):
                    first = True
                    for m in range(4):
                        for ic in range(ICH):
                            nc.tensor.matmul(
                                pp[:, mcc, :],
                                PROJ_MATS[m][:, ic, mcc * 128:(mcc + 1) * 128],
                                xT_all[:, g, ic::2, 3 - m::4],
                                start=first,
                                stop=(m == 3 and ic == ICH - 1),
                            )
                            first = False
                # epilogue adds the radix-4 bias b(I+A+A^2+A^3)
                for mcc in range(HC):
                    nc.scalar.activation(
                        vhalf[half][:, mcc, row * 128:(row + 1) * 128],
                        pp[:, mcc, :],
                        ACT_IDENT,
                        bias=c4[:, mcc, :],
                    )

            def tree_level(src, n_in, S_m, name, sub=None, dst=None):
                """One binary level over b-major cols: src [128, HC, n_in]
                -> dst [128, HC, n_in//2].  sub=(lo,hi) restricts batch
                rows (to keep a 512-col level's PSUM in one bank)."""
                b_lo, b_hi = sub if sub else (0, BC)
                nb = b_hi - b_lo
                per = n_in // BC
                srcr = src[:].rearrange("p c (b j) -> p c b j", b=BC)
                ps = tlpool.tile([128, HC, 128], f32, tag="tl")
                w = nb * per // 2
                for mcc in range(HC):
                    for kc in range(HC):
                        nc.tensor.matmul(
                            ps[:, mcc, 0:w],
                            S_m[:, kc, mcc * 128:(mcc + 1) * 128],
                            srcr[:, kc, b_lo:b_hi, 0::2],
                            start=(kc == 0),
                            stop=(kc == HC - 1),
                        )
                if dst is None:
                    dst = vpool.tile(
                        [128, HC, n_in // 2], bf16, tag=name, name=name
                    )
                dstr = dst[:].rearrange("p c (b j) -> p c b j", b=BC)
                nc.vector.tensor_add(
                    dstr[:, :, b_lo:b_hi, :],
                    ps[:, :, 0:w].rearrange("p c (b j) -> p c b j", b=nb),
                    srcr[:, :, b_lo:b_hi, 1::2],
                )
                return dst

            def emit_tree_half(half):
                v0 = vhalf[half]
                v1 = tree_level(v0, 512, S4, f"v1h{half}", sub=(0, 2))
                tree_level(v0, 512, S4, f"v1h{half}", sub=(2, 4), dst=v1)
                v2 = tree_level(v1, 256, S8, f"v2h{half}")
                v3 = tree_level(v2, 128, S16, f"v3h{half}")
                v4 = tree_level(v3, 64, S32, f"v4h{half}")
                v5[half] = tree_level(v4, 32, S64, f"v5h{half}")

            hs = {}

            def emit_horner(r):
                """h_r = h_{r-1} @ A^128 + V_r (V_r = row-block r%4 of
                half r//4); round 7 writes the f32 output tile."""
                vsrc = v5[r // 4][:].rearrange("p c (b j) -> p c b j", b=BC)
                rhs = (
                    v5[0][:].rearrange("p c (b j) -> p c b j", b=BC)
                    if r == 1 else hs[r - 1][:]
                )
                ps = smpool.tile([128, HC, 4], f32, tag="sm", name="hsm")
                for mcc in range(HC):
                    for kc in range(HC):
                        nc.tensor.matmul(
                            ps[:, mcc, :],
                            S128[:, kc, mcc * 128:(mcc + 1) * 128],
                            rhs[:, kc, :, 0] if r == 1 else rhs[:, kc, :],
                            start=(kc == 0),
                            stop=(kc == HC - 1),
                        )
                if r < T // 128 - 1:
                    h = vpool.tile(
                        [128, HC, BC], bf16, tag="h", bufs=3, name="h"
                    )
                    nc.vector.tensor_add(h[:], ps[:], vsrc[:, :, :, r % 4])
                    hs[r] = h
                else:
                    fout = cpool.tile([128, HC, BC], f32, tag="fout")
                    nc.vector.tensor_add(fout[:], ps[:], vsrc[:, :, :, r % 4])
                    hs[r] = fout

            # half 0: projection + tree
            for g in (0, 2, 4, 6):
                emit_proj(g)
            emit_tree_half(0)
            # second-half x path + projections, with the first Horner
            # rounds (blocks 1-3, all in half 0) interleaved between them
            for g in (1, 3, 5, 7):
                emit_conv(g)
            for g in (1, 3, 5, 7):
                emit_xpose(g)
            emit_proj(1)
            emit_horner(1)
            emit_proj(3)
            emit_horner(2)
            emit_proj(5)
            emit_horner(3)
            emit_proj(7)
            emit_tree_half(1)
            for r in range(4, 8):
                emit_horner(r)

            nc.sync.dma_start(
                out_d.rearrange("p (c b) -> p c b", b=BC), hs[7][:]
            )

    nc.compile()
    return nc


def _get_nc():
    if "nc" not in _cache:
        _cache["nc"] = _build()
    return _cache["nc"]


def _in_maps(inputs):
    x = np.ascontiguousarray(np.asarray(inputs["x"], dtype=np.float32))
    wxh = np.ascontiguousarray(np.asarray(inputs["Wxh"], dtype=np.float32))
    bxh = np.ascontiguousarray(np.asarray(inputs["bxh"], dtype=np.float32))
    whh = np.ascontiguousarray(np.asarray(inputs["Whh"], dtype=np.float32))
    return [
        dict(
            x=np.ascontiguousarray(
                x[c * BC:(c + 1) * BC].reshape(COLS, IN)
            ),
            Wxh=wxh,
            bxh=bxh,
            Whh=whh,
        )
        for c in range(NCORES)
    ]


def kernel(**inputs) -> np.ndarray:
    from concourse.bass_utils import run_bass_kernel_spmd

    res = run_bass_kernel_spmd(
        _get_nc(), _in_maps(inputs), list(range(NCORES))
    ).results
    return _assemble(res)


def _assemble(results) -> np.ndarray:
    outs = []
    for c in range(NCORES):
        o = np.asarray(results[c]["h_out"])      # [128, HC*BC] on-chip layout
        o = o.reshape(128, HC, BC).transpose(2, 1, 0).reshape(BC, H)
        outs.append(o)
    return np.concatenate(outs, axis=0).astype(np.float32)


# revision 17
# speedup vs baseline: 1.2419x; 1.0258x over previous
"""LinearRNN final-state kernel for 8 Trainium2 NeuronCores.

Reference computation:
    u_t = Wxh @ x_t + bxh            (input projection)
    h_t = u_t + Whh @ h_{t-1}        (recurrence over T=1024 steps)
    return h_T                        -> [B=32, H=512]

The recurrence is linear:  h_T = sum_t u_t @ A^(T-1-t),  A = Whh^T
(row-vector convention).  Structure (driven by the ~58ns/instruction PE
floor: matmuls below ~139 moving columns are instruction-bound):

  * radix-4 fused projection: 4 consecutive timesteps fold directly into
    the input GEMM via W, WA, WA^2, WA^3 (W = Wxh^T), absorbing the first
    two tree levels; the bias enters here as b(I+A+A^2+A^3).
  * wide per-half tree levels with A^4..A^64 collapse each half of the
    sequence to 4 blocks of 128 timesteps per batch row (matmuls stay
    >=64 wide);
  * a Horner chain over the 8 blocks with stationary A^128:
    h <- h @ A^128 + V_k.  Rounds 1-3 (first-half blocks) hide behind the
    second half's projection work.
  * only A^2..A^128 are materialized (7 squaring products); their
    lhsT-layout transposes ride the DMA xbar engine (14ns per 16x128
    tile) except the first two, which sit on the setup critical path and
    use PE quad-transposes.

Everything on the PE runs in bf16 (1 cycle/row at any width; empirically
rel-err ~6e-3 vs the 2e-2 budget).  x is converted f32->bf16 on Act and
transposed by the DMA xbar, so the PE does no transpose work for x.

Sharding: data-parallel over batch (B=32 -> 4 rows/core on 8 cores);
weights and the squaring chain are replicated.
"""

import numpy as np

B, T, IN, H = 32, 1024, 256, 512
NCORES = 8
BC = B // NCORES          # 4 batch rows per core
COLS = BC * T             # 4096 sequence columns per core
HC = H // 128             # 4 hidden-dim chunks of 128
ICH = IN // 128           # 2 input-dim chunks
NG = COLS // 512          # 8 x-groups (each = half of one batch row)

_cache: dict = {}


def _build():
    import concourse.bass as bass
    import concourse.mybir as mybir
    from concourse import bacc
    from concourse.tile import TileContext
    from concourse.masks import make_identity

    f32 = mybir.dt.float32
    bf16 = mybir.dt.bfloat16
    ACT_COPY = mybir.ActivationFunctionType.Copy
    ACT_IDENT = mybir.ActivationFunctionType.Identity

    nc = bacc.Bacc(None)
    x_d = nc.declare_dram_parameter("x", [COLS, IN], f32, isOutput=False)
    wxh_d = nc.declare_dram_parameter("Wxh", [H, IN], f32, isOutput=False)
    bxh_d = nc.declare_dram_parameter("bxh", [H], f32, isOutput=False)
    whh_d = nc.declare_dram_parameter("Whh", [H, H], f32, isOutput=False)
    # Output stays in on-chip layout [128, HC*BC]; host unscrambles.
    out_d = nc.declare_dram_parameter("h_out", [128, HC * BC], f32, isOutput=True)

    with TileContext(nc) as tc:
        with (
            tc.tile_pool(name="const", bufs=1) as cpool,
            tc.tile_pool(name="xload", bufs=4) as xpool,
            tc.tile_pool(name="vbuf", bufs=1) as vpool,
            tc.tile_pool(name="mm", bufs=2, space="PSUM") as mmpool,
            tc.tile_pool(name="tr", bufs=1, space="PSUM") as trpool,
            tc.tile_pool(name="pj", bufs=2, space="PSUM") as pjpool,
            tc.tile_pool(name="tl", bufs=2, space="PSUM") as tlpool,
            tc.tile_pool(name="sm", bufs=1, space="PSUM") as smpool,
        ):
            ident_b = cpool.tile([128, 128], bf16, tag="identb")
            make_identity(nc, ident_b[:])

            # PE warm-up: keeps the PE busy through the initial weight-DMA
            # wait and completes the clock ramp before real work arrives.
            warm = mmpool.tile([128, H], f32, tag="mm")
            for _ in range(40):
                nc.tensor.matmul(
                    warm[:, 0:128], ident_b[:], ident_b[:], start=True, stop=True
                )

            # ---- loads -------------------------------------------------
            w_f32 = cpool.tile([128, HC, H], f32, tag="wf32")
            nc.scalar.dma_start(w_f32[:], whh_d.rearrange("(c p) f -> p c f", p=128))
            wxh_f32 = cpool.tile([128, HC, IN], f32, tag="wxf32")
            nc.scalar.dma_start(
                wxh_f32[:], wxh_d.rearrange("(c p) f -> p c f", p=128)
            )
            b_f32 = cpool.tile([128, HC], f32, tag="bf32")
            nc.scalar.dma_start(b_f32[:], bxh_d.rearrange("(c p) -> p c", p=128))

            # x groups; group g = half (g%2) of batch row g//2.  The first
            # half of every row is needed first.
            load_order = [0, 2, 4, 6, 1, 3, 5, 7]
            xg_f32 = {}
            for g in load_order:
                xg = xpool.tile([128, 4, IN], f32, tag="xg")
                nc.sync.dma_start(
                    xg[:],
                    x_d[g * 512:(g + 1) * 512, :].rearrange(
                        "(j p) i -> p j i", p=128
                    ),
                )
                xg_f32[g] = xg

            # ---- bf16 conversions of weights --------------------------
            w_bf = cpool.tile([128, HC, H], bf16, tag="wbf")  # Whh = A^T natural
            for c in range(HC):
                if c % 2:
                    nc.scalar.activation(
                        w_bf[:, c, :], w_f32[:, c, :], ACT_COPY
                    )
                else:
                    nc.vector.tensor_copy(w_bf[:, c, :], w_f32[:, c, :])
            wxh_bf = cpool.tile([128, HC, IN], bf16, tag="wxbf")
            nc.scalar.activation(wxh_bf[:], wxh_f32[:], ACT_COPY)
            b_bf = cpool.tile([128, HC, 1], bf16, tag="bbf")
            nc.scalar.activation(b_bf[:, :, 0], b_f32[:], ACT_COPY)

            # ---- x path: convert to bf16 (Act), transpose via DMA xbar -
            # xT_all[q, g, gb, p] = x_bf[512 g + (gb//2)*128 + p,
            #                            (gb%2)*128 + q]
            xT_all = cpool.tile([128, NG, 8, 128], bf16, tag="xT")
            xg_bf = {}

            def emit_conv(g):
                xb = xpool.tile([128, 4, IN], bf16, tag="xgbf")
                nc.scalar.activation(xb[:], xg_f32[g][:], ACT_COPY)
                xg_bf[g] = xb

            def emit_xpose(g):
                nc.sync.dma_start_transpose(xT_all[:, g, :, :], xg_bf[g][:])

            for g in (0, 2, 4, 6):
                emit_conv(g)
            for g in (0, 2, 4, 6):
                emit_xpose(g)

            def transpose_quad(dst_ap, srcs):
                """PE transpose of up to four [128,128] bf16 blocks through
                one PSUM tile + a single wide DVE copy."""
                tp = trpool.tile([128, 4, 128], bf16, tag="tp")
                for i, s in enumerate(srcs):
                    nc.tensor.transpose(tp[:, i, :], s, ident_b[:])
                nc.vector.tensor_copy(dst_ap, tp[:, :len(srcs), :])

            # S1 = A natural: S1[p, kc, f] = A[kc*128+p, f] = Whh[f, kc*128+p]
            S1 = cpool.tile([128, HC, H], bf16, tag="S1")
            for kc in range(HC):
                transpose_quad(
                    S1[:, kc, :],
                    [w_bf[:, rc, kc * 128:(kc + 1) * 128] for rc in range(HC)],
                )
            # Wq = Wxh^T as lhsT: Wq[p, ic, h] = Wxh[h, ic*128+p]
            Wq = cpool.tile([128, ICH, H], bf16, tag="Wq")
            for ic in range(ICH):
                transpose_quad(
                    Wq[:, ic, :],
                    [wxh_bf[:, rc, ic * 128:(ic + 1) * 128] for rc in range(HC)],
                )

            # lhsT accessors: natural [128, HC, H] tiles slice
            # [:, kc, mcc-block]; DMA-transposed tiles are [128, 16, 128]
            # and slice [:, 4*mcc + kc, :].
            def nat(tile):
                return lambda kc, mcc: tile[:, kc, mcc * 128:(mcc + 1) * 128]

            def xbar(tile):
                return lambda kc, mcc: tile[:, 4 * mcc + kc, :]

            def square(U_sl, S, name, copy_eng="dve"):
                """A^{2m}: out[:, mcc, :] = sum_kc U_sl(kc,mcc)^T @ S[:,kc,:]."""
                Snew = cpool.tile([128, HC, H], bf16, tag=name)
                for mcc in range(HC):
                    ps = mmpool.tile([128, H], f32, tag="mm")
                    for kc in range(HC):
                        nc.tensor.matmul(
                            ps[:],
                            U_sl(kc, mcc),
                            S[:, kc, :],
                            start=(kc == 0),
                            stop=(kc == HC - 1),
                        )
                    if copy_eng == "act":
                        nc.scalar.activation(Snew[:, mcc, :], ps[:], ACT_COPY)
                    else:
                        nc.vector.tensor_copy(Snew[:, mcc, :], ps[:])
                return Snew

            def pe_transpose_mat(S, name):
                U = cpool.tile([128, HC, H], bf16, tag=name)
                for kc in range(HC):
                    transpose_quad(
                        U[:, kc, :],
                        [S[:, fc, kc * 128:(kc + 1) * 128] for fc in range(HC)],
                    )
                return nat(U)

            def dma_transpose_mat(S, name):
                U = cpool.tile([128, 16, 128], bf16, tag=name)
                nc.scalar.dma_start_transpose(U[:], S[:])
                return xbar(U)

            def c_round(c_prev, S_m, name):
                """c_{2m} = c_m + c_m @ A^m  (column form)."""
                psf = smpool.tile([128, HC, 4], f32, tag="sm", name="csm")
                ps = psf[:, :, 0:1]
                for mcc in range(HC):
                    for kc in range(HC):
                        nc.tensor.matmul(
                            ps[:, mcc, :],
                            S_m[:, kc, mcc * 128:(mcc + 1) * 128],
                            c_prev[:, kc, :],
                            start=(kc == 0),
                            stop=(kc == HC - 1),
                        )
                c_new = cpool.tile([128, HC, 1], bf16, tag=name)
                nc.vector.tensor_add(c_new[:], ps[:], c_prev[:])
                return c_new

            def g_mats(lhs_sl, rhs, name):
                """G[:, oc, :] = sum_kc lhs_sl(kc,oc)^T @ rhs(kc), 512 wide."""
                G = cpool.tile([128, ICH, H], bf16, tag=name)
                for oc in range(ICH):
                    ps = mmpool.tile([128, H], f32, tag="mm")
                    for kc in range(HC):
                        nc.tensor.matmul(
                            ps[:],
                            lhs_sl(kc, oc),
                            rhs(kc),
                            start=(kc == 0),
                            stop=(kc == HC - 1),
                        )
                    nc.scalar.activation(G[:, oc, :], ps[:], ACT_COPY)
                return G

            # ---- squaring chain / G matrices / bias vector -------------
            S2 = square(nat(w_bf), S1, "S2")        # U1 = Whh natural
            c2 = c_round(b_bf, S1, "c2")
            # G1 = W A  (lhsT chunks = Wxh natural rows of wxh_bf)
            G1 = g_mats(
                lambda kc, oc: wxh_bf[:, kc, oc * 128:(oc + 1) * 128],
                lambda kc: S1[:, kc, :], "G1",
            )
            U2 = pe_transpose_mat(S2, "U2")
            c4 = c_round(c2, S2, "c4")              # = b(I+A+A^2+A^3)
            G2 = g_mats(
                lambda kc, oc: wxh_bf[:, kc, oc * 128:(oc + 1) * 128],
                lambda kc: S2[:, kc, :], "G2",
            )
            S4 = square(U2, S2, "S4")
            # G1T[p, ·, ·] = G1 transposed via DMA xbar (for G3)
            G1T = cpool.tile([128, 8, 128], bf16, tag="G1T")
            nc.scalar.dma_start_transpose(G1T[:], G1[:])
            U4 = pe_transpose_mat(S4, "U4")
            # G3 = G1 A^2 = W A^3
            G3 = g_mats(
                lambda kc, oc: G1T[:, 4 * oc + kc, :],
                lambda kc: S2[:, kc, :], "G3",
            )
            S8 = square(U4, S4, "S8")
            U8 = dma_transpose_mat(S8, "U8")

            PROJ_MATS = [Wq, G1, G2, G3]  # applied to t ≡ 3,2,1,0 (mod 4)

            # ---- main pipeline -----------------------------------------
            # Per x-group (512 timesteps of one batch row): radix-4 fused
            # projection -> 128 quad-columns.  Per half (4 groups, one per
            # row): tree levels A^4..A^64 collapse 512 -> 16 columns
            # (4 blocks of 128 timesteps x 4 rows, b-major).  A Horner
            # chain joins the 8 blocks with A^128.
            vhalf = {}
            v5 = {}

            def emit_proj(g):
                half, row = g % 2, g // 2
                if half not in vhalf:
                    vhalf[half] = vpool.tile(
                        [128, HC, 512], bf16, tag=f"v0h{half}",
                        name=f"v0h{half}",
                    )
                pp = pjpool.tile([128, HC, 128], f32, tag="pj")
                for mcc in range(HC):
                    first = True
                    for m in range(4):
                        for ic in range(ICH):
                            nc.tensor.matmul(
                                pp[:, mcc, :],
                                PROJ_MATS[m][:, ic, mcc * 128:(mcc + 1) * 128],
                                xT_all[:, g, ic::2, 3 - m::4],
                                start=first,
                                stop=(m == 3 and ic == ICH - 1),
                            )
                            first = False
                # epilogue adds the radix-4 bias b(I+A+A^2+A^3)
                for mcc in range(HC):
                    nc.scalar.activation(
                        vhalf[half][:, mcc, row * 128:(row + 1) * 128],
                        pp[:, mcc, :],
                        ACT_IDENT,
                        bias=c4[:, mcc, :],
                    )

            def tree_level(src, n_in, S_m, name, sub=None, dst=None):
                """One binary level over b-major cols: src [128, HC, n_in]
                -> dst [128, HC, n_in//2].  sub=(lo,hi) restricts batch
                rows (to keep a 512-col level's PSUM in one bank)."""
                b_lo, b_hi = sub if sub else (0, BC)
                nb = b_hi - b_lo
                per = n_in // BC
                srcr = src[:].rearrange("p c (b j) -> p c b j", b=BC)
                ps = tlpool.tile([128, HC, 128], f32, tag="tl")
                w = nb * per // 2
                for mcc in range(HC):
                    for kc in range(HC):
                        nc.tensor.matmul(
                            ps[:, mcc, 0:w],
                            S_m[:, kc, mcc * 128:(mcc + 1) * 128],
                            srcr[:, kc, b_lo:b_hi, 0::2],
                            start=(kc == 0),
                            stop=(kc == HC - 1),
                        )
                if dst is None:
                    dst = vpool.tile(
                        [128, HC, n_in // 2], bf16, tag=name, name=name
                    )
                dstr = dst[:].rearrange("p c (b j) -> p c b j", b=BC)
                nc.vector.tensor_add(
                    dstr[:, :, b_lo:b_hi, :],
                    ps[:, :, 0:w].rearrange("p c (b j) -> p c b j", b=nb),
                    srcr[:, :, b_lo:b_hi, 1::2],
                )
                return dst

            def emit_tree_half(half):
                v0 = vhalf[half]
                v1 = tree_level(v0, 512, S4, f"v1h{half}", sub=(0, 2))
                tree_level(v0, 512, S4, f"v1h{half}", sub=(2, 4), dst=v1)
                v2 = tree_level(v1, 256, S8, f"v2h{half}")
                v3 = tree_level(v2, 128, S16, f"v3h{half}")
                v4 = tree_level(v3, 64, S32, f"v4h{half}")
                v5[half] = tree_level(v4, 32, S64, f"v5h{half}")

            hs = {}

            def emit_horner(r):
                """h_r = h_{r-1} @ A^128 + V_r (V_r = row-block r%4 of
                half r//4); round 7 writes the f32 output tile."""
                vsrc = v5[r // 4][:].rearrange("p c (b j) -> p c b j", b=BC)
                rhs = (
                    v5[0][:].rearrange("p c (b j) -> p c b j", b=BC)
                    if r == 1 else hs[r - 1][:]
                )
                ps = smpool.tile([128, HC, 4], f32, tag="sm", name="hsm")
                for mcc in range(HC):
                    for kc in range(HC):
                        nc.tensor.matmul(
                            ps[:, mcc, :],
                            S128[:, kc, mcc * 128:(mcc + 1) * 128],
                            rhs[:, kc, :, 0] if r == 1 else rhs[:, kc, :],
                            start=(kc == 0),
                            stop=(kc == HC - 1),
                        )
                if r < T // 128 - 1:
                    h = vpool.tile(
                        [128, HC, BC], bf16, tag="h", bufs=3, name="h"
                    )
                    nc.vector.tensor_add(h[:], ps[:], vsrc[:, :, :, r % 4])
                    hs[r] = h
                else:
                    fout = cpool.tile([128, HC, BC], f32, tag="fout")
                    nc.vector.tensor_add(fout[:], ps[:], vsrc[:, :, :, r % 4])
                    hs[r] = fout

            # half 0 projections and tree levels interleave with the
            # tail of the squaring chain: each DMA-transposed U lands
            # while the PE chews a chunk of main-phase work.
            emit_proj(0)
            emit_proj(2)
            S16 = square(U8, S8, "S16")
            U16 = dma_transpose_mat(S16, "U16")
            emit_proj(4)
            emit_proj(6)
            S32 = square(U16, S16, "S32")
            U32 = dma_transpose_mat(S32, "U32")
            v0h = vhalf[0]
            v1h = tree_level(v0h, 512, S4, "v1h0", sub=(0, 2))
            tree_level(v0h, 512, S4, "v1h0", sub=(2, 4), dst=v1h)
            S64 = square(U32, S32, "S64")
            U64 = dma_transpose_mat(S64, "U64")
            v2h = tree_level(v1h, 256, S8, "v2h0")
            v3h = tree_level(v2h, 128, S16, "v3h0")
            S128 = square(U64, S64, "S128", copy_eng="act")
            v4h = tree_level(v3h, 64, S32, "v4h0")
            v5[0] = tree_level(v4h, 32, S64, "v5h0")
            # second-half x path + projections, with the first Horner
            # rounds (blocks 1-3, all in half 0) interleaved between them
            for g in (1, 3, 5, 7):
                emit_conv(g)
            for g in (1, 3, 5, 7):
                emit_xpose(g)
            emit_proj(1)
            emit_horner(1)
            emit_proj(3)
            emit_horner(2)
            emit_proj(5)
            emit_horner(3)
            emit_proj(7)
            emit_tree_half(1)
            for r in range(4, 8):
                emit_horner(r)

            nc.sync.dma_start(
                out_d.rearrange("p (c b) -> p c b", b=BC), hs[7][:]
            )

    nc.compile()
    return nc


def _get_nc():
    if "nc" not in _cache:
        _cache["nc"] = _build()
    return _cache["nc"]


def _in_maps(inputs):
    x = np.ascontiguousarray(np.asarray(inputs["x"], dtype=np.float32))
    wxh = np.ascontiguousarray(np.asarray(inputs["Wxh"], dtype=np.float32))
    bxh = np.ascontiguousarray(np.asarray(inputs["bxh"], dtype=np.float32))
    whh = np.ascontiguousarray(np.asarray(inputs["Whh"], dtype=np.float32))
    return [
        dict(
            x=np.ascontiguousarray(
                x[c * BC:(c + 1) * BC].reshape(COLS, IN)
            ),
            Wxh=wxh,
            bxh=bxh,
            Whh=whh,
        )
        for c in range(NCORES)
    ]


def kernel(**inputs) -> np.ndarray:
    from concourse.bass_utils import run_bass_kernel_spmd

    res = run_bass_kernel_spmd(
        _get_nc(), _in_maps(inputs), list(range(NCORES))
    ).results
    return _assemble(res)


def _assemble(results) -> np.ndarray:
    outs = []
    for c in range(NCORES):
        o = np.asarray(results[c]["h_out"])      # [128, HC*BC] on-chip layout
        o = o.reshape(128, HC, BC).transpose(2, 1, 0).reshape(BC, H)
        outs.append(o)
    return np.concatenate(outs, axis=0).astype(np.float32)


# revision 18
# speedup vs baseline: 1.3078x; 1.0530x over previous
"""LinearRNN final-state kernel for 8 Trainium2 NeuronCores.

Reference computation:
    u_t = Wxh @ x_t + bxh            (input projection)
    h_t = u_t + Whh @ h_{t-1}        (recurrence over T=1024 steps)
    return h_T                        -> [B=32, H=512]

The recurrence is linear:  h_T = sum_t u_t @ A^(T-1-t),  A = Whh^T
(row-vector convention).  Structure (driven by the ~58ns/instruction PE
floor: matmuls below ~139 moving columns are instruction-bound):

  * radix-4 fused projection: 4 consecutive timesteps fold directly into
    the input GEMM via W, WA, WA^2, WA^3 (W = Wxh^T), absorbing the first
    two tree levels; the bias enters here as b(I+A+A^2+A^3).
  * wide per-half tree levels with A^4..A^64 collapse each half of the
    sequence to 4 blocks of 128 timesteps per batch row (matmuls stay
    >=64 wide);
  * a Horner chain over the 8 blocks with stationary A^128:
    h <- h @ A^128 + V_k.  Rounds 1-3 (first-half blocks) hide behind the
    second half's projection work.
  * only A^2..A^128 are materialized (7 squaring products); their
    lhsT-layout transposes ride the DMA xbar engine (14ns per 16x128
    tile) except the first two, which sit on the setup critical path and
    use PE quad-transposes.

Everything on the PE runs in bf16 (1 cycle/row at any width; empirically
rel-err ~6e-3 vs the 2e-2 budget).  x is converted f32->bf16 on Act and
transposed by the DMA xbar, so the PE does no transpose work for x.

Sharding: data-parallel over batch (B=32 -> 4 rows/core on 8 cores);
weights and the squaring chain are replicated.
"""

import numpy as np

B, T, IN, H = 32, 1024, 256, 512
NCORES = 8
BC = B // NCORES          # 4 batch rows per core
COLS = BC * T             # 4096 sequence columns per core
HC = H // 128             # 4 hidden-dim chunks of 128
ICH = IN // 128           # 2 input-dim chunks
NG = COLS // 512          # 8 x-groups (each = half of one batch row)

_cache: dict = {}


def _build():
    import concourse.bass as bass
    import concourse.mybir as mybir
    from concourse import bacc
    from concourse.tile import TileContext
    from concourse.masks import make_identity

    f32 = mybir.dt.float32
    bf16 = mybir.dt.bfloat16
    ACT_COPY = mybir.ActivationFunctionType.Copy
    ACT_IDENT = mybir.ActivationFunctionType.Identity

    nc = bacc.Bacc(None)
    x_d = nc.declare_dram_parameter("x", [COLS, IN], f32, isOutput=False)
    wxh_d = nc.declare_dram_parameter("Wxh", [H, IN], f32, isOutput=False)
    bxh_d = nc.declare_dram_parameter("bxh", [H], f32, isOutput=False)
    whh_d = nc.declare_dram_parameter("Whh", [H, H], f32, isOutput=False)
    # Output stays in on-chip layout [128, HC*BC]; host unscrambles.
    out_d = nc.declare_dram_parameter("h_out", [128, HC * BC], f32, isOutput=True)

    with TileContext(nc) as tc:
        with (
            tc.tile_pool(name="const", bufs=1) as cpool,
            tc.tile_pool(name="xload", bufs=4) as xpool,
            tc.tile_pool(name="vbuf", bufs=1) as vpool,
            tc.tile_pool(name="mm", bufs=2, space="PSUM") as mmpool,
            tc.tile_pool(name="tr", bufs=1, space="PSUM") as trpool,
            tc.tile_pool(name="pj", bufs=2, space="PSUM") as pjpool,
            tc.tile_pool(name="tl", bufs=2, space="PSUM") as tlpool,
            tc.tile_pool(name="sm", bufs=1, space="PSUM") as smpool,
        ):
            ident_b = cpool.tile([128, 128], bf16, tag="identb")
            make_identity(nc, ident_b[:])

            # PE warm-up: keeps the PE busy through the initial weight-DMA
            # wait and completes the clock ramp before real work arrives.
            warm = mmpool.tile([128, H], f32, tag="mm")
            for _ in range(40):
                nc.tensor.matmul(
                    warm[:, 0:128], ident_b[:], ident_b[:], start=True, stop=True
                )

            # ---- loads -------------------------------------------------
            w_f32 = cpool.tile([128, HC, H], f32, tag="wf32")
            nc.scalar.dma_start(w_f32[:], whh_d.rearrange("(c p) f -> p c f", p=128))
            wxh_f32 = cpool.tile([128, HC, IN], f32, tag="wxf32")
            nc.scalar.dma_start(
                wxh_f32[:], wxh_d.rearrange("(c p) f -> p c f", p=128)
            )
            b_f32 = cpool.tile([128, HC], f32, tag="bf32")
            nc.scalar.dma_start(b_f32[:], bxh_d.rearrange("(c p) -> p c", p=128))

            # x groups; group g = half (g%2) of batch row g//2.  The first
            # half of every row is needed first.
            load_order = [0, 2, 4, 6, 1, 3, 5, 7]
            xg_f32 = {}
            for g in load_order:
                xg = xpool.tile([128, 4, IN], f32, tag="xg")
                nc.sync.dma_start(
                    xg[:],
                    x_d[g * 512:(g + 1) * 512, :].rearrange(
                        "(j p) i -> p j i", p=128
                    ),
                )
                xg_f32[g] = xg

            # ---- bf16 conversions of weights --------------------------
            w_bf = cpool.tile([128, HC, H], bf16, tag="wbf")  # Whh = A^T natural
            for c in range(HC):
                if c % 2:
                    nc.scalar.activation(
                        w_bf[:, c, :], w_f32[:, c, :], ACT_COPY
                    )
                else:
                    nc.vector.tensor_copy(w_bf[:, c, :], w_f32[:, c, :])
            wxh_bf = cpool.tile([128, HC, IN], bf16, tag="wxbf")
            nc.scalar.activation(wxh_bf[:], wxh_f32[:], ACT_COPY)
            # all x conversions up front on the Act ring: their DMAs all
            # land by ~12us and everything downstream queues behind them
            # on this in-order engine.
            b_bf = cpool.tile([128, HC, 1], bf16, tag="bbf")
            nc.scalar.activation(b_bf[:, :, 0], b_f32[:], ACT_COPY)

            # ---- x path: convert to bf16 (Act), transpose via DMA xbar -
            # xT_all[q, g, gb, p] = x_bf[512 g + (gb//2)*128 + p,
            #                            (gb%2)*128 + q]
            xT_all = cpool.tile([128, NG, 8, 128], bf16, tag="xT")
            xg_bf = {}

            def emit_conv(g):
                xb = xpool.tile([128, 4, IN], bf16, tag="xgbf")
                nc.scalar.activation(xb[:], xg_f32[g][:], ACT_COPY)
                xg_bf[g] = xb

            def emit_xpose(g):
                nc.sync.dma_start_transpose(xT_all[:, g, :, :], xg_bf[g][:])

            for g in (0, 2, 4, 6, 1, 3, 5, 7):
                emit_conv(g)
            for g in (0, 2, 4, 6, 1, 3, 5, 7):
                emit_xpose(g)

            def transpose_quad(dst_ap, srcs):
                """PE transpose of up to four [128,128] bf16 blocks through
                one PSUM tile + a single wide DVE copy."""
                tp = trpool.tile([128, 4, 128], bf16, tag="tp")
                for i, s in enumerate(srcs):
                    nc.tensor.transpose(tp[:, i, :], s, ident_b[:])
                nc.vector.tensor_copy(dst_ap, tp[:, :len(srcs), :])

            # S1 = A natural: S1[p, kc, f] = A[kc*128+p, f] = Whh[f, kc*128+p]
            S1 = cpool.tile([128, HC, H], bf16, tag="S1")
            for kc in range(HC):
                transpose_quad(
                    S1[:, kc, :],
                    [w_bf[:, rc, kc * 128:(kc + 1) * 128] for rc in range(HC)],
                )
            # Wq = Wxh^T as lhsT: Wq[p, ic, h] = Wxh[h, ic*128+p]
            Wq = cpool.tile([128, ICH, H], bf16, tag="Wq")
            for ic in range(ICH):
                transpose_quad(
                    Wq[:, ic, :],
                    [wxh_bf[:, rc, ic * 128:(ic + 1) * 128] for rc in range(HC)],
                )

            # lhsT accessors: natural [128, HC, H] tiles slice
            # [:, kc, mcc-block]; DMA-transposed tiles are [128, 16, 128]
            # and slice [:, 4*mcc + kc, :].
            def nat(tile):
                return lambda kc, mcc: tile[:, kc, mcc * 128:(mcc + 1) * 128]

            def xbar(tile):
                return lambda kc, mcc: tile[:, 4 * mcc + kc, :]

            def square(U_sl, S, name, copy_eng="dve"):
                """A^{2m}: out[:, mcc, :] = sum_kc U_sl(kc,mcc)^T @ S[:,kc,:]."""
                Snew = cpool.tile([128, HC, H], bf16, tag=name)
                for mcc in range(HC):
                    ps = mmpool.tile([128, H], f32, tag="mm")
                    for kc in range(HC):
                        nc.tensor.matmul(
                            ps[:],
                            U_sl(kc, mcc),
                            S[:, kc, :],
                            start=(kc == 0),
                            stop=(kc == HC - 1),
                        )
                    if copy_eng == "act":
                        nc.scalar.activation(Snew[:, mcc, :], ps[:], ACT_COPY)
                    else:
                        nc.vector.tensor_copy(Snew[:, mcc, :], ps[:])
                return Snew

            def pe_transpose_mat(S, name):
                U = cpool.tile([128, HC, H], bf16, tag=name)
                for kc in range(HC):
                    transpose_quad(
                        U[:, kc, :],
                        [S[:, fc, kc * 128:(kc + 1) * 128] for fc in range(HC)],
                    )
                return nat(U)

            def dma_transpose_mat(S, name):
                U = cpool.tile([128, 16, 128], bf16, tag=name)
                nc.sync.dma_start_transpose(U[:], S[:])
                return xbar(U)

            def c_round(c_prev, S_m, name):
                """c_{2m} = c_m + c_m @ A^m  (column form)."""
                psf = smpool.tile([128, HC, 4], f32, tag="sm", name="csm")
                ps = psf[:, :, 0:1]
                for mcc in range(HC):
                    for kc in range(HC):
                        nc.tensor.matmul(
                            ps[:, mcc, :],
                            S_m[:, kc, mcc * 128:(mcc + 1) * 128],
                            c_prev[:, kc, :],
                            start=(kc == 0),
                            stop=(kc == HC - 1),
                        )
                c_new = cpool.tile([128, HC, 1], bf16, tag=name)
                nc.vector.tensor_add(c_new[:], ps[:], c_prev[:])
                return c_new

            def g_mats(lhs_sl, rhs, name):
                """G[:, oc, :] = sum_kc lhs_sl(kc,oc)^T @ rhs(kc), 512 wide."""
                G = cpool.tile([128, ICH, H], bf16, tag=name)
                for oc in range(ICH):
                    ps = mmpool.tile([128, H], f32, tag="mm")
                    for kc in range(HC):
                        nc.tensor.matmul(
                            ps[:],
                            lhs_sl(kc, oc),
                            rhs(kc),
                            start=(kc == 0),
                            stop=(kc == HC - 1),
                        )
                    nc.scalar.activation(G[:, oc, :], ps[:], ACT_COPY)
                return G

            # ---- squaring chain / G matrices / bias vector -------------
            S2 = square(nat(w_bf), S1, "S2")        # U1 = Whh natural
            c2 = c_round(b_bf, S1, "c2")
            # G1 = W A  (lhsT chunks = Wxh natural rows of wxh_bf)
            G1 = g_mats(
                lambda kc, oc: wxh_bf[:, kc, oc * 128:(oc + 1) * 128],
                lambda kc: S1[:, kc, :], "G1",
            )
            U2 = pe_transpose_mat(S2, "U2")
            c4 = c_round(c2, S2, "c4")              # = b(I+A+A^2+A^3)
            G2 = g_mats(
                lambda kc, oc: wxh_bf[:, kc, oc * 128:(oc + 1) * 128],
                lambda kc: S2[:, kc, :], "G2",
            )
            S4 = square(U2, S2, "S4")
            # G1T[p, ·, ·] = G1 transposed via DMA xbar (for G3)
            G1T = cpool.tile([128, 8, 128], bf16, tag="G1T")
            nc.sync.dma_start_transpose(G1T[:], G1[:])
            U4 = pe_transpose_mat(S4, "U4")
            # G3 = G1 A^2 = W A^3
            G3 = g_mats(
                lambda kc, oc: G1T[:, 4 * oc + kc, :],
                lambda kc: S2[:, kc, :], "G3",
            )
            S8 = square(U4, S4, "S8")
            U8 = dma_transpose_mat(S8, "U8")

            PROJ_MATS = [Wq, G1, G2, G3]  # applied to t ≡ 3,2,1,0 (mod 4)

            # ---- main pipeline -----------------------------------------
            # Per x-group (512 timesteps of one batch row): radix-4 fused
            # projection -> 128 quad-columns.  Per half (4 groups, one per
            # row): tree levels A^4..A^64 collapse 512 -> 16 columns
            # (4 blocks of 128 timesteps x 4 rows, b-major).  A Horner
            # chain joins the 8 blocks with A^128.
            vhalf = {}
            v5 = {}

            def emit_proj(g):
                half, row = g % 2, g // 2
                if half not in vhalf:
                    vhalf[half] = vpool.tile(
                        [128, HC, 512], bf16, tag=f"v0h{half}",
                        name=f"v0h{half}",
                    )
                pp = pjpool.tile([128, HC, 128], f32, tag="pj")
                for mcc in range(HC):
                    first = True
                    for m in range(4):
                        for ic in range(ICH):
                            nc.tensor.matmul(
                                pp[:, mcc, :],
                                PROJ_MATS[m][:, ic, mcc * 128:(mcc + 1) * 128],
                                xT_all[:, g, ic::2, 3 - m::4],
                                start=first,
                                stop=(m == 3 and ic == ICH - 1),
                            )
                            first = False
                # epilogue adds the radix-4 bias b(I+A+A^2+A^3)
                for mcc in range(HC):
                    nc.scalar.activation(
                        vhalf[half][:, mcc, row * 128:(row + 1) * 128],
                        pp[:, mcc, :],
                        ACT_IDENT,
                        bias=c4[:, mcc, :],
                    )

            def tree_level(src, n_in, S_m, name, sub=None, dst=None):
                """One binary level over b-major cols: src [128, HC, n_in]
                -> dst [128, HC, n_in//2].  sub=(lo,hi) restricts batch
                rows (to keep a 512-col level's PSUM in one bank)."""
                b_lo, b_hi = sub if sub else (0, BC)
                nb = b_hi - b_lo
                per = n_in // BC
                srcr = src[:].rearrange("p c (b j) -> p c b j", b=BC)
                ps = tlpool.tile([128, HC, 128], f32, tag="tl")
                w = nb * per // 2
                for mcc in range(HC):
                    for kc in range(HC):
                        nc.tensor.matmul(
                            ps[:, mcc, 0:w],
                            S_m[:, kc, mcc * 128:(mcc + 1) * 128],
                            srcr[:, kc, b_lo:b_hi, 0::2],
                            start=(kc == 0),
                            stop=(kc == HC - 1),
                        )
                if dst is None:
                    dst = vpool.tile(
                        [128, HC, n_in // 2], bf16, tag=name, name=name
                    )
                dstr = dst[:].rearrange("p c (b j) -> p c b j", b=BC)
                nc.vector.tensor_add(
                    dstr[:, :, b_lo:b_hi, :],
                    ps[:, :, 0:w].rearrange("p c (b j) -> p c b j", b=nb),
                    srcr[:, :, b_lo:b_hi, 1::2],
                )
                return dst

            def emit_tree_half(half):
                v0 = vhalf[half]
                v1 = tree_level(v0, 512, S4, f"v1h{half}", sub=(0, 2))
                tree_level(v0, 512, S4, f"v1h{half}", sub=(2, 4), dst=v1)
                v2 = tree_level(v1, 256, S8, f"v2h{half}")
                v3 = tree_level(v2, 128, S16, f"v3h{half}")
                v4 = tree_level(v3, 64, S32, f"v4h{half}")
                v5[half] = tree_level(v4, 32, S64, f"v5h{half}")

            hs = {}

            def emit_horner(r):
                """h_r = h_{r-1} @ A^128 + V_r (V_r = row-block r%4 of
                half r//4); round 7 writes the f32 output tile."""
                vsrc = v5[r // 4][:].rearrange("p c (b j) -> p c b j", b=BC)
                rhs = (
                    v5[0][:].rearrange("p c (b j) -> p c b j", b=BC)
                    if r == 1 else hs[r - 1][:]
                )
                ps = smpool.tile([128, HC, 4], f32, tag="sm", name="hsm")
                for mcc in range(HC):
                    for kc in range(HC):
                        nc.tensor.matmul(
                            ps[:, mcc, :],
                            S128[:, kc, mcc * 128:(mcc + 1) * 128],
                            rhs[:, kc, :, 0] if r == 1 else rhs[:, kc, :],
                            start=(kc == 0),
                            stop=(kc == HC - 1),
                        )
                if r < T // 128 - 1:
                    h = vpool.tile(
                        [128, HC, BC], bf16, tag="h", bufs=3, name="h"
                    )
                    nc.vector.tensor_add(h[:], ps[:], vsrc[:, :, :, r % 4])
                    hs[r] = h
                else:
                    fout = cpool.tile([128, HC, BC], f32, tag="fout")
                    nc.vector.tensor_add(fout[:], ps[:], vsrc[:, :, :, r % 4])
                    hs[r] = fout

            # half 0 projections and tree levels interleave with the
            # tail of the squaring chain: each DMA-transposed U lands
            # while the PE chews a chunk of main-phase work.
            emit_proj(0)
            emit_proj(2)
            S16 = square(U8, S8, "S16")
            U16 = dma_transpose_mat(S16, "U16")
            emit_proj(4)
            emit_proj(6)
            S32 = square(U16, S16, "S32")
            U32 = dma_transpose_mat(S32, "U32")
            v0h = vhalf[0]
            v1h = tree_level(v0h, 512, S4, "v1h0", sub=(0, 2))
            tree_level(v0h, 512, S4, "v1h0", sub=(2, 4), dst=v1h)
            S64 = square(U32, S32, "S64")
            U64 = dma_transpose_mat(S64, "U64")
            v2h = tree_level(v1h, 256, S8, "v2h0")
            v3h = tree_level(v2h, 128, S16, "v3h0")
            S128 = square(U64, S64, "S128", copy_eng="act")
            v4h = tree_level(v3h, 64, S32, "v4h0")
            v5[0] = tree_level(v4h, 32, S64, "v5h0")
            # second-half projections, with the first Horner rounds
            # (blocks 1-3, all in half 0) interleaved between them
            emit_proj(1)
            emit_horner(1)
            emit_proj(3)
            emit_horner(2)
            emit_proj(5)
            emit_horner(3)
            emit_proj(7)
            emit_tree_half(1)
            for r in range(4, 8):
                emit_horner(r)

            nc.sync.dma_start(
                out_d.rearrange("p (c b) -> p c b", b=BC), hs[7][:]
            )

    nc.compile()
    return nc


def _get_nc():
    if "nc" not in _cache:
        _cache["nc"] = _build()
    return _cache["nc"]


def _in_maps(inputs):
    x = np.ascontiguousarray(np.asarray(inputs["x"], dtype=np.float32))
    wxh = np.ascontiguousarray(np.asarray(inputs["Wxh"], dtype=np.float32))
    bxh = np.ascontiguousarray(np.asarray(inputs["bxh"], dtype=np.float32))
    whh = np.ascontiguousarray(np.asarray(inputs["Whh"], dtype=np.float32))
    return [
        dict(
            x=np.ascontiguousarray(
                x[c * BC:(c + 1) * BC].reshape(COLS, IN)
            ),
            Wxh=wxh,
            bxh=bxh,
            Whh=whh,
        )
        for c in range(NCORES)
    ]


def kernel(**inputs) -> np.ndarray:
    from concourse.bass_utils import run_bass_kernel_spmd

    res = run_bass_kernel_spmd(
        _get_nc(), _in_maps(inputs), list(range(NCORES))
    ).results
    return _assemble(res)


def _assemble(results) -> np.ndarray:
    outs = []
    for c in range(NCORES):
        o = np.asarray(results[c]["h_out"])      # [128, HC*BC] on-chip layout
        o = o.reshape(128, HC, BC).transpose(2, 1, 0).reshape(BC, H)
        outs.append(o)
    return np.concatenate(outs, axis=0).astype(np.float32)


# revision 19
# speedup vs baseline: 1.3119x; 1.0031x over previous
"""LinearRNN final-state kernel for 8 Trainium2 NeuronCores.

Reference computation:
    u_t = Wxh @ x_t + bxh            (input projection)
    h_t = u_t + Whh @ h_{t-1}        (recurrence over T=1024 steps)
    return h_T                        -> [B=32, H=512]

The recurrence is linear:  h_T = sum_t u_t @ A^(T-1-t),  A = Whh^T
(row-vector convention).  Structure (driven by the ~58ns/instruction PE
floor: matmuls below ~139 moving columns are instruction-bound):

  * radix-4 fused projection: 4 consecutive timesteps fold directly into
    the input GEMM via W, WA, WA^2, WA^3 (W = Wxh^T), absorbing the first
    two tree levels; the bias enters here as b(I+A+A^2+A^3).
  * wide per-half tree levels with A^4..A^64 collapse each half of the
    sequence to 4 blocks of 128 timesteps per batch row (matmuls stay
    >=64 wide);
  * a Horner chain over the 8 blocks with stationary A^128:
    h <- h @ A^128 + V_k.  Rounds 1-3 (first-half blocks) hide behind the
    second half's projection work.
  * only A^2..A^128 are materialized (7 squaring products); their
    lhsT-layout transposes ride the DMA xbar engine (14ns per 16x128
    tile) except the first two, which sit on the setup critical path and
    use PE quad-transposes.

Everything on the PE runs in bf16 (1 cycle/row at any width; empirically
rel-err ~6e-3 vs the 2e-2 budget).  x is converted f32->bf16 on Act and
transposed by the DMA xbar, so the PE does no transpose work for x.

Sharding: data-parallel over batch (B=32 -> 4 rows/core on 8 cores);
weights and the squaring chain are replicated.
"""

import numpy as np

B, T, IN, H = 32, 1024, 256, 512
NCORES = 8
BC = B // NCORES          # 4 batch rows per core
COLS = BC * T             # 4096 sequence columns per core
HC = H // 128             # 4 hidden-dim chunks of 128
ICH = IN // 128           # 2 input-dim chunks
NG = COLS // 512          # 8 x-groups (each = half of one batch row)

_cache: dict = {}


def _build():
    import concourse.bass as bass
    import concourse.mybir as mybir
    from concourse import bacc
    from concourse.tile import TileContext
    from concourse.masks import make_identity

    f32 = mybir.dt.float32
    bf16 = mybir.dt.bfloat16
    ACT_COPY = mybir.ActivationFunctionType.Copy
    ACT_IDENT = mybir.ActivationFunctionType.Identity

    nc = bacc.Bacc(None)
    x_d = nc.declare_dram_parameter("x", [COLS, IN], f32, isOutput=False)
    wxh_d = nc.declare_dram_parameter("Wxh", [H, IN], f32, isOutput=False)
    bxh_d = nc.declare_dram_parameter("bxh", [H], f32, isOutput=False)
    whh_d = nc.declare_dram_parameter("Whh", [H, H], f32, isOutput=False)
    # Output stays in on-chip layout [128, HC*BC]; host unscrambles.
    out_d = nc.declare_dram_parameter("h_out", [128, HC * BC], f32, isOutput=True)

    with TileContext(nc) as tc:
        with (
            tc.tile_pool(name="const", bufs=1) as cpool,
            tc.tile_pool(name="xload", bufs=4) as xpool,
            tc.tile_pool(name="vbuf", bufs=1) as vpool,
            tc.tile_pool(name="mm", bufs=2, space="PSUM") as mmpool,
            tc.tile_pool(name="tr", bufs=1, space="PSUM") as trpool,
            tc.tile_pool(name="pj", bufs=2, space="PSUM") as pjpool,
            tc.tile_pool(name="tl", bufs=2, space="PSUM") as tlpool,
            tc.tile_pool(name="sm", bufs=1, space="PSUM") as smpool,
        ):
            ident_b = cpool.tile([128, 128], bf16, tag="identb")
            make_identity(nc, ident_b[:])

            # PE warm-up: keeps the PE busy through the initial weight-DMA
            # wait and completes the clock ramp before real work arrives.
            warm = mmpool.tile([128, H], f32, tag="mm")
            for _ in range(48):
                nc.tensor.matmul(
                    warm[:, 0:128], ident_b[:], ident_b[:], start=True, stop=True
                )

            # ---- loads -------------------------------------------------
            w_f32 = cpool.tile([128, HC, H], f32, tag="wf32")
            nc.scalar.dma_start(w_f32[:], whh_d.rearrange("(c p) f -> p c f", p=128))
            wxh_f32 = cpool.tile([128, HC, IN], f32, tag="wxf32")
            nc.scalar.dma_start(
                wxh_f32[:], wxh_d.rearrange("(c p) f -> p c f", p=128)
            )
            b_f32 = cpool.tile([128, HC], f32, tag="bf32")
            nc.scalar.dma_start(b_f32[:], bxh_d.rearrange("(c p) -> p c", p=128))

            # x groups; group g = half (g%2) of batch row g//2.  The first
            # half of every row is needed first.
            load_order = [0, 2, 4, 6, 1, 3, 5, 7]
            xg_f32 = {}
            for g in load_order:
                xg = xpool.tile([128, 4, IN], f32, tag="xg")
                nc.sync.dma_start(
                    xg[:],
                    x_d[g * 512:(g + 1) * 512, :].rearrange(
                        "(j p) i -> p j i", p=128
                    ),
                )
                xg_f32[g] = xg

            # ---- bf16 conversions of weights --------------------------
            w_bf = cpool.tile([128, HC, H], bf16, tag="wbf")  # Whh = A^T natural
            for c in range(HC):
                if c % 2:
                    nc.scalar.activation(
                        w_bf[:, c, :], w_f32[:, c, :], ACT_COPY
                    )
                else:
                    nc.vector.tensor_copy(w_bf[:, c, :], w_f32[:, c, :])
            wxh_bf = cpool.tile([128, HC, IN], bf16, tag="wxbf")
            nc.scalar.activation(wxh_bf[:], wxh_f32[:], ACT_COPY)
            # all x conversions up front on the Act ring: their DMAs all
            # land by ~12us and everything downstream queues behind them
            # on this in-order engine.
            b_bf = cpool.tile([128, HC, 1], bf16, tag="bbf")
            nc.scalar.activation(b_bf[:, :, 0], b_f32[:], ACT_COPY)

            # ---- x path: convert to bf16 (Act), transpose via DMA xbar -
            # xT_all[q, g, gb, p] = x_bf[512 g + (gb//2)*128 + p,
            #                            (gb%2)*128 + q]
            xT_all = cpool.tile([128, NG, 8, 128], bf16, tag="xT")
            xg_bf = {}

            def emit_conv(g):
                xb = xpool.tile([128, 4, IN], bf16, tag="xgbf")
                nc.scalar.activation(xb[:], xg_f32[g][:], ACT_COPY)
                xg_bf[g] = xb

            def emit_xpose(g):
                nc.sync.dma_start_transpose(xT_all[:, g, :, :], xg_bf[g][:])

            for g in (0, 2, 4, 6, 1, 3, 5, 7):
                emit_conv(g)
            for g in (0, 2, 4, 6, 1, 3, 5, 7):
                emit_xpose(g)

            def transpose_quad(dst_ap, srcs):
                """PE transpose of up to four [128,128] bf16 blocks through
                one PSUM tile + a single wide DVE copy."""
                tp = trpool.tile([128, 4, 128], bf16, tag="tp")
                for i, s in enumerate(srcs):
                    nc.tensor.transpose(tp[:, i, :], s, ident_b[:])
                nc.vector.tensor_copy(dst_ap, tp[:, :len(srcs), :])

            # S1 = A natural: S1[p, kc, f] = A[kc*128+p, f] = Whh[f, kc*128+p]
            S1 = cpool.tile([128, HC, H], bf16, tag="S1")
            for kc in range(HC):
                transpose_quad(
                    S1[:, kc, :],
                    [w_bf[:, rc, kc * 128:(kc + 1) * 128] for rc in range(HC)],
                )
            # Wq = Wxh^T as lhsT: Wq[p, ic, h] = Wxh[h, ic*128+p]
            Wq = cpool.tile([128, ICH, H], bf16, tag="Wq")
            for ic in range(ICH):
                transpose_quad(
                    Wq[:, ic, :],
                    [wxh_bf[:, rc, ic * 128:(ic + 1) * 128] for rc in range(HC)],
                )

            # lhsT accessors: natural [128, HC, H] tiles slice
            # [:, kc, mcc-block]; DMA-transposed tiles are [128, 16, 128]
            # and slice [:, 4*mcc + kc, :].
            def nat(tile):
                return lambda kc, mcc: tile[:, kc, mcc * 128:(mcc + 1) * 128]

            def xbar(tile):
                return lambda kc, mcc: tile[:, 4 * mcc + kc, :]

            def square(U_sl, S, name, copy_eng="dve"):
                """A^{2m}: out[:, mcc, :] = sum_kc U_sl(kc,mcc)^T @ S[:,kc,:]."""
                Snew = cpool.tile([128, HC, H], bf16, tag=name)
                for mcc in range(HC):
                    ps = mmpool.tile([128, H], f32, tag="mm")
                    for kc in range(HC):
                        nc.tensor.matmul(
                            ps[:],
                            U_sl(kc, mcc),
                            S[:, kc, :],
                            start=(kc == 0),
                            stop=(kc == HC - 1),
                        )
                    if copy_eng == "act":
                        nc.scalar.activation(Snew[:, mcc, :], ps[:], ACT_COPY)
                    else:
                        nc.vector.tensor_copy(Snew[:, mcc, :], ps[:])
                return Snew

            def pe_transpose_mat(S, name):
                U = cpool.tile([128, HC, H], bf16, tag=name)
                for kc in range(HC):
                    transpose_quad(
                        U[:, kc, :],
                        [S[:, fc, kc * 128:(kc + 1) * 128] for fc in range(HC)],
                    )
                return nat(U)

            def dma_transpose_mat(S, name):
                U = cpool.tile([128, 16, 128], bf16, tag=name)
                nc.sync.dma_start_transpose(U[:], S[:])
                return xbar(U)

            def c_round(c_prev, S_m, name):
                """c_{2m} = c_m + c_m @ A^m  (column form)."""
                psf = smpool.tile([128, HC, 4], f32, tag="sm", name="csm")
                ps = psf[:, :, 0:1]
                for mcc in range(HC):
                    for kc in range(HC):
                        nc.tensor.matmul(
                            ps[:, mcc, :],
                            S_m[:, kc, mcc * 128:(mcc + 1) * 128],
                            c_prev[:, kc, :],
                            start=(kc == 0),
                            stop=(kc == HC - 1),
                        )
                c_new = cpool.tile([128, HC, 1], bf16, tag=name)
                nc.vector.tensor_add(c_new[:], ps[:], c_prev[:])
                return c_new

            def g_mats(lhs_sl, rhs, name):
                """G[:, oc, :] = sum_kc lhs_sl(kc,oc)^T @ rhs(kc), 512 wide."""
                G = cpool.tile([128, ICH, H], bf16, tag=name)
                for oc in range(ICH):
                    ps = mmpool.tile([128, H], f32, tag="mm")
                    for kc in range(HC):
                        nc.tensor.matmul(
                            ps[:],
                            lhs_sl(kc, oc),
                            rhs(kc),
                            start=(kc == 0),
                            stop=(kc == HC - 1),
                        )
                    nc.scalar.activation(G[:, oc, :], ps[:], ACT_COPY)
                return G

            # ---- squaring chain / G matrices / bias vector -------------
            S2 = square(nat(w_bf), S1, "S2")        # U1 = Whh natural
            c2 = c_round(b_bf, S1, "c2")
            # G1 = W A  (lhsT chunks = Wxh natural rows of wxh_bf)
            G1 = g_mats(
                lambda kc, oc: wxh_bf[:, kc, oc * 128:(oc + 1) * 128],
                lambda kc: S1[:, kc, :], "G1",
            )
            U2 = pe_transpose_mat(S2, "U2")
            c4 = c_round(c2, S2, "c4")              # = b(I+A+A^2+A^3)
            G2 = g_mats(
                lambda kc, oc: wxh_bf[:, kc, oc * 128:(oc + 1) * 128],
                lambda kc: S2[:, kc, :], "G2",
            )
            S4 = square(U2, S2, "S4")
            # G1T[p, ·, ·] = G1 transposed via DMA xbar (for G3)
            G1T = cpool.tile([128, 8, 128], bf16, tag="G1T")
            nc.sync.dma_start_transpose(G1T[:], G1[:])
            U4 = pe_transpose_mat(S4, "U4")
            # G3 = G1 A^2 = W A^3
            G3 = g_mats(
                lambda kc, oc: G1T[:, 4 * oc + kc, :],
                lambda kc: S2[:, kc, :], "G3",
            )
            S8 = square(U4, S4, "S8")
            U8 = dma_transpose_mat(S8, "U8")

            PROJ_MATS = [Wq, G1, G2, G3]  # applied to t ≡ 3,2,1,0 (mod 4)

            # ---- main pipeline -----------------------------------------
            # Per x-group (512 timesteps of one batch row): radix-4 fused
            # projection -> 128 quad-columns.  Per half (4 groups, one per
            # row): tree levels A^4..A^64 collapse 512 -> 16 columns
            # (4 blocks of 128 timesteps x 4 rows, b-major).  A Horner
            # chain joins the 8 blocks with A^128.
            vhalf = {}
            v5 = {}

            def emit_proj(g):
                half, row = g % 2, g // 2
                if half not in vhalf:
                    vhalf[half] = vpool.tile(
                        [128, HC, 512], bf16, tag=f"v0h{half}",
                        name=f"v0h{half}",
                    )
                pp = pjpool.tile([128, HC, 128], f32, tag="pj")
                for mcc in range(HC):
                    first = True
                    for m in range(4):
                        for ic in range(ICH):
                            nc.tensor.matmul(
                                pp[:, mcc, :],
                                PROJ_MATS[m][:, ic, mcc * 128:(mcc + 1) * 128],
                                xT_all[:, g, ic::2, 3 - m::4],
                                start=first,
                                stop=(m == 3 and ic == ICH - 1),
                            )
                            first = False
                # epilogue adds the radix-4 bias b(I+A+A^2+A^3)
                for mcc in range(HC):
                    nc.scalar.activation(
                        vhalf[half][:, mcc, row * 128:(row + 1) * 128],
                        pp[:, mcc, :],
                        ACT_IDENT,
                        bias=c4[:, mcc, :],
                    )

            def tree_level(src, n_in, S_m, name, sub=None, dst=None):
                """One binary level over b-major cols: src [128, HC, n_in]
                -> dst [128, HC, n_in//2].  sub=(lo,hi) restricts batch
                rows (to keep a 512-col level's PSUM in one bank)."""
                b_lo, b_hi = sub if sub else (0, BC)
                nb = b_hi - b_lo
                per = n_in // BC
                srcr = src[:].rearrange("p c (b j) -> p c b j", b=BC)
                ps = tlpool.tile([128, HC, 128], f32, tag="tl")
                w = nb * per // 2
                for mcc in range(HC):
                    for kc in range(HC):
                        nc.tensor.matmul(
                            ps[:, mcc, 0:w],
                            S_m[:, kc, mcc * 128:(mcc + 1) * 128],
                            srcr[:, kc, b_lo:b_hi, 0::2],
                            start=(kc == 0),
                            stop=(kc == HC - 1),
                        )
                if dst is None:
                    dst = vpool.tile(
                        [128, HC, n_in // 2], bf16, tag=name, name=name
                    )
                dstr = dst[:].rearrange("p c (b j) -> p c b j", b=BC)
                nc.vector.tensor_add(
                    dstr[:, :, b_lo:b_hi, :],
                    ps[:, :, 0:w].rearrange("p c (b j) -> p c b j", b=nb),
                    srcr[:, :, b_lo:b_hi, 1::2],
                )
                return dst

            def emit_tree_half(half):
                v0 = vhalf[half]
                v1 = tree_level(v0, 512, S4, f"v1h{half}", sub=(0, 2))
                tree_level(v0, 512, S4, f"v1h{half}", sub=(2, 4), dst=v1)
                v2 = tree_level(v1, 256, S8, f"v2h{half}")
                v3 = tree_level(v2, 128, S16, f"v3h{half}")
                v4 = tree_level(v3, 64, S32, f"v4h{half}")
                v5[half] = tree_level(v4, 32, S64, f"v5h{half}")

            hs = {}

            def emit_horner(r):
                """h_r = h_{r-1} @ A^128 + V_r (V_r = row-block r%4 of
                half r//4); round 7 writes the f32 output tile."""
                vsrc = v5[r // 4][:].rearrange("p c (b j) -> p c b j", b=BC)
                rhs = (
                    v5[0][:].rearrange("p c (b j) -> p c b j", b=BC)
                    if r == 1 else hs[r - 1][:]
                )
                ps = smpool.tile([128, HC, 4], f32, tag="sm", name="hsm")
                for mcc in range(HC):
                    for kc in range(HC):
                        nc.tensor.matmul(
                            ps[:, mcc, :],
                            S128[:, kc, mcc * 128:(mcc + 1) * 128],
                            rhs[:, kc, :, 0] if r == 1 else rhs[:, kc, :],
                            start=(kc == 0),
                            stop=(kc == HC - 1),
                        )
                if r < T // 128 - 1:
                    h = vpool.tile(
                        [128, HC, BC], bf16, tag="h", bufs=3, name="h"
                    )
                    nc.vector.tensor_add(h[:], ps[:], vsrc[:, :, :, r % 4])
                    hs[r] = h
                else:
                    fout = cpool.tile([128, HC, BC], f32, tag="fout")
                    nc.vector.tensor_add(fout[:], ps[:], vsrc[:, :, :, r % 4])
                    hs[r] = fout

            # Projections and tree levels interleave with the tail of
            # the squaring chain: each DMA-transposed U lands while the
            # PE chews a chunk of main-phase work, and S128's matmuls are
            # saved for last so they can pad the half-1 tree ladder.
            emit_proj(0)
            emit_proj(2)
            S16 = square(U8, S8, "S16")
            U16 = dma_transpose_mat(S16, "U16")
            emit_proj(4)
            emit_proj(6)
            S32 = square(U16, S16, "S32")
            U32 = dma_transpose_mat(S32, "U32")
            v0h = vhalf[0]
            v1h = tree_level(v0h, 512, S4, "v1h0", sub=(0, 2))
            tree_level(v0h, 512, S4, "v1h0", sub=(2, 4), dst=v1h)
            S64 = square(U32, S32, "S64")
            U64 = dma_transpose_mat(S64, "U64")
            v2h = tree_level(v1h, 256, S8, "v2h0")
            emit_proj(1)
            v3h = tree_level(v2h, 128, S16, "v3h0")
            emit_proj(3)
            v4h = tree_level(v3h, 64, S32, "v4h0")
            emit_proj(5)
            v5[0] = tree_level(v4h, 32, S64, "v5h0")
            emit_proj(7)
            S128 = square(U64, S64, "S128", copy_eng="act")
            # half-1 tree ladder with the early Horner rounds (blocks
            # 1-3, all in half 0) interleaved to hide the round trips
            v0h1 = vhalf[1]
            v1h1 = tree_level(v0h1, 512, S4, "v1h1", sub=(0, 2))
            emit_horner(1)
            tree_level(v0h1, 512, S4, "v1h1", sub=(2, 4), dst=v1h1)
            emit_horner(2)
            v2h1 = tree_level(v1h1, 256, S8, "v2h1")
            emit_horner(3)
            v3h1 = tree_level(v2h1, 128, S16, "v3h1")
            v4h1 = tree_level(v3h1, 64, S32, "v4h1")
            v5[1] = tree_level(v4h1, 32, S64, "v5h1")
            for r in range(4, 8):
                emit_horner(r)

            nc.sync.dma_start(
                out_d.rearrange("p (c b) -> p c b", b=BC), hs[7][:]
            )

    nc.compile()
    return nc


def _get_nc():
    if "nc" not in _cache:
        _cache["nc"] = _build()
    return _cache["nc"]


def _in_maps(inputs):
    x = np.ascontiguousarray(np.asarray(inputs["x"], dtype=np.float32))
    wxh = np.ascontiguousarray(np.asarray(inputs["Wxh"], dtype=np.float32))
    bxh = np.ascontiguousarray(np.asarray(inputs["bxh"], dtype=np.float32))
    whh = np.ascontiguousarray(np.asarray(inputs["Whh"], dtype=np.float32))
    return [
        dict(
            x=np.ascontiguousarray(
                x[c * BC:(c + 1) * BC].reshape(COLS, IN)
            ),
            Wxh=wxh,
            bxh=bxh,
            Whh=whh,
        )
        for c in range(NCORES)
    ]


def kernel(**inputs) -> np.ndarray:
    from concourse.bass_utils import run_bass_kernel_spmd

    res = run_bass_kernel_spmd(
        _get_nc(), _in_maps(inputs), list(range(NCORES))
    ).results
    return _assemble(res)


def _assemble(results) -> np.ndarray:
    outs = []
    for c in range(NCORES):
        o = np.asarray(results[c]["h_out"])      # [128, HC*BC] on-chip layout
        o = o.reshape(128, HC, BC).transpose(2, 1, 0).reshape(BC, H)
        outs.append(o)
    return np.concatenate(outs, axis=0).astype(np.float32)
